# revision 29
# baseline (speedup 1.0000x reference)
"""Trainium2 Bass kernel for a GPT-style transformer block.

Shapes (hardcoded): x [2, 2048, 1024], n_head=16, causal attention + GELU MLP.
Strategy: row-sharding (4096 rows -> 512 rows/core on 8 cores).
  Launch A: per-core LN1 + qkv projection for own rows -> qkvT [3072, 512] bf16.
  Host:     reassemble full K^T / V per batch; per-core SLOT PERMUTATION of the
            16 key tiles so the 4 diagonal tiles sit at fixed slots 0-3 (their
            triangular masks are then core-independent constants), remaining
            slots carry a per-core {0,-1e9} exp bias that zeroes fully-masked
            tiles for free inside the activation.
  Launch B: per-core attention over own 512 query rows, proj, residual, LN2,
            FFN, residual -> out rows [512, 1024].
All matmuls bf16 with fp32 PSUM accumulation; residual stream / LN / softmax
sums fp32. Attention computes scores for head pairs row-packed on the PE
(K=64 halves at tile_position (0,0)/(64,0)) into one [128, 1024] PSUM span so
exp runs on big tiles (scalar ACTIVATE cost is (N+352)/1.2ns, N = free size).
"""

import sys

if "/opt/trn_rl_repo" not in sys.path:
    sys.path.insert(0, "/opt/trn_rl_repo")

import numpy as np
import ml_dtypes

import concourse.bacc as bacc
import concourse.tile as tile
from concourse import masks
from concourse import mybir
from concourse.bass_utils import run_bass_kernel_spmd

F32 = mybir.dt.float32
BF16 = mybir.dt.bfloat16
AF = mybir.ActivationFunctionType
ALU = mybir.AluOpType
BF = ml_dtypes.bfloat16

B, T, D = 2, 2048, 1024
H, DH = 16, 64
R = B * T          # 4096 flattened rows
NCORE = 8
RC = R // NCORE    # 512 rows per core
EPS = 1e-5
D3 = 3 * D         # 3072
DF = 4 * D         # 4096
VA = H * (DH + 1)  # 1040: V augmented with a ones column per head
NKT = T // 128     # 16 key tiles
NDIAG = 4          # diagonal key tiles per core (512 q rows / 128)
NEG = -1.0e9


def _slot_order(qb128):
    """Key-tile processing order for a core whose queries start at 128*qb128.
    Slots 0..3 = the diagonal tiles; slots 4..15 = the rest ascending."""
    diag = list(range(qb128, qb128 + NDIAG))
    rest = [k for k in range(NKT) if k not in diag]
    return diag + rest


def _layer_norm_tiles(nc, sb, x_tile, out_dtype):
    """Plain LN (no scale/shift: gamma/beta are folded into the downstream
    weights host-side) over free axis of x_tile [128, D] fp32 -> [128, D]."""
    stats = sb.tile([128, 2, 6], F32, tag="ln_stats")
    xg = x_tile[:].rearrange("p (s d) -> p s d", s=2)
    for s in range(2):
        nc.vector.bn_stats(stats[:, s, :], xg[:, s, :])
    mv = sb.tile([128, 2], F32, tag="ln_mv")
    nc.vector.bn_aggr(mv[:], stats[:])
    rstd = sb.tile([128, 1], F32, tag="ln_rstd")
    eps_sb = sb.tile([128, 1], F32, tag="ln_eps")
    nc.vector.memset(eps_sb[:], EPS)
    nc.scalar.activation(rstd[:], mv[:, 1:2], AF.Sqrt, bias=eps_sb[:], scale=1.0)
    nc.vector.reciprocal(rstd[:], rstd[:])
    out = sb.tile([128, D], out_dtype, tag="ln_out")
    nc.vector.tensor_scalar(
        out[:], x_tile[:], mv[:, 0:1], rstd[:], ALU.subtract, ALU.mult,
    )
    return out


def _pe_transpose(nc, tps, dst_tiles, src_tile, t, ident):
    """Transpose src [128, D] bf16 into dst_tiles[j][:, t*128:(t+1)*128]
    via the PE transpose path + copy out of PSUM (split DVE/gpsimd)."""
    for j in range(8):
        tp = tps.tile([128, 128], BF16, tag="tp")
        nc.tensor.transpose(tp[:], src_tile[:, j * 128:(j + 1) * 128], ident[:])
        nc.vector.tensor_copy(dst_tiles[j][:, t * 128:(t + 1) * 128], tp[:])


def _build_a():
    nc = bacc.Bacc("TRN2", target_bir_lowering=False, debug=False, num_devices=NCORE)
    x = nc.dram_tensor("x", [RC, D], F32, kind="ExternalInput")
    wattn = nc.dram_tensor("wattn", [D, D3], BF16, kind="ExternalInput")
    battn = nc.dram_tensor("battn", [D3], F32, kind="ExternalInput")
    qkvT = nc.dram_tensor("qkvT", [D3, RC], BF16, kind="ExternalOutput")

    with tile.TileContext(nc) as tc:
        with (
            tc.tile_pool(name="const", bufs=1) as const,
            tc.tile_pool(name="w", bufs=1) as wpool,
            tc.tile_pool(name="sb", bufs=2) as sb,
            tc.tile_pool(name="lt", bufs=1) as lt,
            tc.tile_pool(name="ps", bufs=4, space="PSUM") as ps,
            tc.tile_pool(name="tps", bufs=3, space="PSUM") as tps,
            tc.tile_pool(name="outp", bufs=3) as outp,
        ):
            # x first (critical path: LN1 -> transpose -> matmul); weights
            # staggered on sync/scalar rings behind it.
            x_sb = [sb.tile([128, D], F32, tag=f"x{t}", name=f"x{t}") for t in range(4)]
            for t in range(4):
                nc.gpsimd.dma_start(x_sb[t][:], x[t * 128:(t + 1) * 128, :])

            ident = const.tile([128, 128], BF16, tag="ident")
            masks.make_identity(nc, ident[:])
            battn_all = const.tile([128, D3 // 128], F32, tag="battn")
            nc.gpsimd.dma_start(battn_all[:], battn.ap().rearrange("(m p) -> p m", p=128))

            w_sb = [wpool.tile([128, D3], BF16, tag=f"w{k}", name=f"w{k}") for k in range(8)]
            for k in range(8):
                eng = nc.sync if k % 2 == 0 else nc.gpsimd
                eng.dma_start(w_sb[k][:], wattn[k * 128:(k + 1) * 128, :])

            ln1T = [lt.tile([128, RC], BF16, tag=f"ln1T{j}", name=f"ln1T{j}") for j in range(8)]
            for t in range(4):
                ln1n = _layer_norm_tiles(nc, sb, x_sb[t], BF16)
                _pe_transpose(nc, tps, ln1T, ln1n, t, ident)

            for m in range(D3 // 128):  # 24 output col-tiles
                psum = ps.tile([128, RC], F32, tag="mm")
                for k in range(8):
                    nc.tensor.matmul(
                        psum[:], w_sb[k][:, m * 128:(m + 1) * 128], ln1T[k][:],
                        start=(k == 0), stop=(k == 7),
                    )
                o_sb = outp.tile([128, RC], BF16, tag="o")
                nc.scalar.activation(
                    o_sb[:], psum[:], AF.Identity,
                    bias=battn_all[:, m:m + 1], scale=1.0,
                )
                nc.sync.dma_start(qkvT[m * 128:(m + 1) * 128, :], o_sb[:])

    nc.compile()
    return nc


def _build_b():
    nc = bacc.Bacc("TRN2", target_bir_lowering=False, debug=False, num_devices=NCORE)
    x = nc.dram_tensor("x", [RC, D], F32, kind="ExternalInput")
    qT = nc.dram_tensor("qT", [D, RC], BF16, kind="ExternalInput")
    kT = nc.dram_tensor("kT", [D, T], BF16, kind="ExternalInput")          # slot-permuted
    vaug = nc.dram_tensor("vaug", [T, VA], BF16, kind="ExternalInput")     # slot-permuted
    biast = nc.dram_tensor("biast", [128, NKT - NDIAG], F32, kind="ExternalInput")
    diagmask = nc.dram_tensor("diagmask", [NDIAG * 128, 2 * RC], BF16, kind="ExternalInput")
    wproj = nc.dram_tensor("wproj", [D, D], BF16, kind="ExternalInput")
    bproj = nc.dram_tensor("bproj", [D], F32, kind="ExternalInput")
    wfc = nc.dram_tensor("wfc", [D, DF], BF16, kind="ExternalInput")
    bfc = nc.dram_tensor("bfc", [DF], F32, kind="ExternalInput")
    wout = nc.dram_tensor("wout", [DF, D], BF16, kind="ExternalInput")
    bout = nc.dram_tensor("bout", [D], F32, kind="ExternalInput")
    out = nc.dram_tensor("out", [RC, D], F32, kind="ExternalOutput")

    with tile.TileContext(nc) as tc:
        with (
            tc.tile_pool(name="const", bufs=1) as const,
            tc.tile_pool(name="att", bufs=1) as attp,
            tc.tile_pool(name="sb", bufs=2) as sb,
        ):
            attT = [attp.tile([128, RC], BF16, tag=f"attT{i}", name=f"attT{i}") for i in range(8)]

            # ---------------- attention ----------------
            with (
                tc.tile_pool(name="kv", bufs=1) as kv,
                tc.tile_pool(name="exps", bufs=20) as exps,
                tc.tile_pool(name="aps", bufs=3, space="PSUM") as aps,
                tc.tile_pool(name="avps", bufs=2, space="PSUM") as avps,
                tc.tile_pool(name="asb", bufs=4) as asb,
            ):
                # critical-path DMAs first: qT/kT per head pair on sync,
                # vaug slots + tiny mask/bias tables early on scalar/gpsimd.
                qT_sb = [kv.tile([128, RC], BF16, tag=f"qT{i}", name=f"qTs{i}") for i in range(8)]
                kT_sb = [kv.tile([128, T], BF16, tag=f"kT{i}", name=f"kTs{i}") for i in range(8)]
                v_sb = [kv.tile([128, VA], BF16, tag=f"v{i}", name=f"vs{i}") for i in range(NKT)]
                dm_sb = [kv.tile([128, 2 * RC], BF16, tag=f"dm{i}", name=f"dms{i}")
                         for i in range(NDIAG)]
                biast_sb = kv.tile([128, NKT - NDIAG], F32, tag="biast")
                nc.gpsimd.dma_start(biast_sb[:], biast[:, :])
                for i in range(NDIAG):
                    nc.gpsimd.dma_start(dm_sb[i][:], diagmask[i * 128:(i + 1) * 128, :])
                for i in range(8):
                    nc.sync.dma_start(qT_sb[i][:], qT[i * 128:(i + 1) * 128, :])
                    nc.sync.dma_start(kT_sb[i][:], kT[i * 128:(i + 1) * 128, :])
                for i in range(NKT):
                    nc.gpsimd.dma_start(v_sb[i][:], vaug[i * 128:(i + 1) * 128, :])

                # weights / residual stream loads overlap the attention phase
                ident = const.tile([128, 128], BF16, tag="ident")
                masks.make_identity(nc, ident[:])
                bproj_bc = const.tile([128, D], F32, tag="bproj")
                nc.gpsimd.dma_start(bproj_bc[:], bproj.ap().partition_broadcast(128))
                bout_bc = const.tile([128, D], F32, tag="bout")
                nc.gpsimd.dma_start(bout_bc[:], bout.ap().partition_broadcast(128))
                bfc_all = const.tile([128, DF // 128], F32, tag="bfc")
                nc.gpsimd.dma_start(bfc_all[:], bfc.ap().rearrange("(m p) -> p m", p=128))

                for hp in range(H // 2):  # head pairs (2hp, 2hp+1)
                    e_tiles = []
                    for s in range(NKT):
                        s_ps = aps.tile([128, 2 * RC], F32, tag="s")
                        nc.tensor.matmul(
                            s_ps[:, 0:RC],
                            kT_sb[hp][0:64, s * 128:(s + 1) * 128],
                            qT_sb[hp][0:64, :],
                            start=True, stop=True, tile_position=(0, 0),
                        )
                        nc.tensor.matmul(
                            s_ps[:, RC:2 * RC],
                            kT_sb[hp][64:128, s * 128:(s + 1) * 128],
                            qT_sb[hp][64:128, :],
                            start=True, stop=True, tile_position=(64, 0),
                        )
                        e_sb = exps.tile([128, 2 * RC], BF16, tag="e")
                        if s < NDIAG:
                            # diagonal tile: plain exp then static triangular mask
                            nc.scalar.activation(e_sb[:], s_ps[:], AF.Exp,
                                                 bias=0.0, scale=0.125)
                            nc.vector.tensor_mul(e_sb[:], e_sb[:], dm_sb[s][:])
                        else:
                            # interior tile: bias is 0 (fully visible) or -1e9
                            # (fully masked -> exp gives exact 0), per-core data
                            nc.scalar.activation(e_sb[:], s_ps[:], AF.Exp,
                                                 bias=biast_sb[:, s - NDIAG:s - NDIAG + 1],
                                                 scale=0.125)
                        e_tiles.append(e_sb)
                    psums = asb.tile([1, 2 * RC], F32, tag="psums")
                    for half in range(2):
                        h = 2 * hp + half
                        hl = half * 64
                        av_ps = avps.tile([DH + 1, RC], F32, tag="av")
                        for s in range(NKT):
                            nc.tensor.matmul(
                                av_ps[:], v_sb[s][:, h * 65:(h + 1) * 65],
                                e_tiles[s][:, half * RC:(half + 1) * RC],
                                start=(s == 0), stop=(s == NKT - 1),
                            )
                        nc.vector.tensor_copy(
                            psums[0:1, half * RC:(half + 1) * RC],
                            av_ps[DH:DH + 1, :])
                        nc.vector.tensor_copy(attT[hp][hl:hl + 64, :], av_ps[0:DH, :])
                    # normalize this pair now; overlaps next pair's QK/exp
                    recips = asb.tile([1, 2 * RC], F32, tag="recips")
                    nc.vector.reciprocal_approx_fast(recips[:], psums[:])
                    r16 = asb.tile([1, 2 * RC], BF16, tag="r16")
                    with nc.allow_low_precision(reason="softmax recip in bf16"):
                        nc.vector.tensor_copy(r16[:], recips[:])
                    rb_sb = asb.tile([128, 2 * RC], BF16, tag="rb")
                    nc.gpsimd.partition_broadcast(rb_sb[:], r16[:])
                    nc.vector.tensor_mul(
                        attT[hp][0:64, :], attT[hp][0:64, :], rb_sb[0:64, 0:RC],
                    )
                    nc.vector.tensor_mul(
                        attT[hp][64:128, :], attT[hp][64:128, :],
                        rb_sb[64:128, RC:2 * RC],
                    )

            # ---------------- proj + residual + LN2 ----------------
            x2p_cm = tc.tile_pool(name="x2p", bufs=1)
            x2p = x2p_cm.__enter__()
            x_sb = [x2p.tile([128, D], F32, tag=f"x{t}", name=f"x{t}") for t in range(4)]
            for t in range(4):
                nc.gpsimd.dma_start(x_sb[t][:], x[t * 128:(t + 1) * 128, :])
                # fold the proj bias into the residual ahead of time (gpsimd,
                # off the critical chain): x_sb becomes x + bproj
                nc.gpsimd.tensor_add(x_sb[t][:], x_sb[t][:], bproj_bc[:])
            ln2T = [x2p.tile([128, RC], BF16, tag=f"ln2T{j}", name=f"ln2T{j}") for j in range(8)]
            x2_sb = [x2p.tile([128, D], F32, tag=f"x2{t}", name=f"x2{t}") for t in range(4)]
            with (
                tc.tile_pool(name="wpj", bufs=1) as wpj,
                tc.tile_pool(name="pps", bufs=3, space="PSUM") as pps,
                tc.tile_pool(name="tps", bufs=3, space="PSUM") as tps,
            ):
                wp_sb = [wpj.tile([128, D], BF16, tag=f"wp{i}", name=f"wp{i}") for i in range(8)]
                for i in range(8):
                    nc.sync.dma_start(wp_sb[i][:], wproj[i * 128:(i + 1) * 128, :])
                # software-pipelined across the 4 query tiles: emit each stage
                # for all qm before the next so no engine queue blocks behind a
                # cross-engine dependency of an earlier qm
                for qm in range(4):
                    for oc in range(2):
                        y_ps = pps.tile([128, 512], F32, tag="y")
                        for cc in range(8):
                            nc.tensor.matmul(
                                y_ps[:],
                                attT[cc][:, qm * 128:(qm + 1) * 128],
                                wp_sb[cc][:, oc * 512:(oc + 1) * 512],
                                start=(cc == 0), stop=(cc == 7),
                            )
                        sl = slice(oc * 512, (oc + 1) * 512)
                        nc.vector.tensor_add(x2_sb[qm][:, sl], y_ps[:], x_sb[qm][:, sl])
                stats = [sb.tile([128, 2, 6], F32, tag=f"ln_stats{qm}", name=f"st{qm}")
                         for qm in range(4)]
                mv = [sb.tile([128, 2], F32, tag=f"ln_mv{qm}", name=f"mv{qm}") for qm in range(4)]
                rstd = [sb.tile([128, 1], F32, tag=f"ln_rstd{qm}", name=f"rs{qm}") for qm in range(4)]
                eps_sb = sb.tile([128, 1], F32, tag="ln_eps")
                nc.vector.memset(eps_sb[:], EPS)
                for qm in range(4):
                    xg = x2_sb[qm][:].rearrange("p (s d) -> p s d", s=2)
                    for s in range(2):
                        nc.vector.bn_stats(stats[qm][:, s, :], xg[:, s, :])
                    nc.vector.bn_aggr(mv[qm][:], stats[qm][:])
                for qm in range(4):
                    nc.scalar.activation(rstd[qm][:], mv[qm][:, 1:2], AF.Sqrt,
                                         bias=eps_sb[:], scale=1.0)
                for qm in range(4):
                    nc.vector.reciprocal(rstd[qm][:], rstd[qm][:])
                ln2n = [sb.tile([128, D], BF16, tag=f"ln_out{qm}", name=f"ln2n{qm}")
                        for qm in range(4)]
                for qm in range(4):
                    nc.vector.tensor_scalar(
                        ln2n[qm][:], x2_sb[qm][:], mv[qm][:, 0:1], rstd[qm][:],
                        ALU.subtract, ALU.mult,
                    )
                for qm in range(4):
                    _pe_transpose(nc, tps, ln2T, ln2n[qm], qm, ident)
                    # pre-bias the FFN residual in place (WAR on LN2 reads is
                    # tracked by the tile framework)
                    nc.gpsimd.tensor_add(x2_sb[qm][:], x2_sb[qm][:], bout_bc[:])

            # ---------------- FFN ----------------
            with tc.tile_pool(name="g", bufs=1) as gp:
                g_sb = [gp.tile([128, RC], BF16, tag=f"g{i}", name=f"g{i}") for i in range(32)]
                woA_cm = tc.tile_pool(name="woA", bufs=1)
                woA = woA_cm.__enter__()
                # first half of w_out preloads while wfc is still resident
                wo_sb = [woA.tile([128, D], BF16, tag=f"wo{i}", name=f"wos{i}")
                         for i in range(16)]
                for i in range(16):
                    eng = nc.sync if i % 2 == 0 else nc.gpsimd
                    eng.dma_start(wo_sb[i][:], wout[i * 128:(i + 1) * 128, :])
                with (
                    tc.tile_pool(name="wf", bufs=1) as wf,
                    tc.tile_pool(name="fps", bufs=4, space="PSUM") as fps,
                ):
                    wf_sb = [wf.tile([128, DF], BF16, tag=f"wf{i}", name=f"wfs{i}") for i in range(8)]
                    for i in range(8):
                        eng = nc.sync if i % 2 == 0 else nc.gpsimd
                        eng.dma_start(wf_sb[i][:], wfc[i * 128:(i + 1) * 128, :])
                    for hm in range(32):
                        h_ps = fps.tile([128, RC], F32, tag="h")
                        for k in range(8):
                            nc.tensor.matmul(
                                h_ps[:], wf_sb[k][:, hm * 128:(hm + 1) * 128],
                                ln2T[k][:], start=(k == 0), stop=(k == 7),
                            )
                        nc.scalar.activation(
                            g_sb[hm][:], h_ps[:], AF.Gelu,
                            bias=bfc_all[:, hm:hm + 1], scale=1.0,
                        )

                with (
                    tc.tile_pool(name="woB", bufs=1) as woB,
                    tc.tile_pool(name="ops", bufs=4, space="PSUM") as ops,
                    tc.tile_pool(name="osb", bufs=3) as osb,
                ):
                    wo_sb += [woB.tile([128, D], BF16, tag=f"wo{i}", name=f"wosb{i}")
                              for i in range(16, 32)]
                    for i in range(16, 32):
                        eng = nc.sync if i % 2 == 0 else nc.gpsimd
                        eng.dma_start(wo_sb[i][:], wout[i * 128:(i + 1) * 128, :])
                    for qm in range(4):
                        o_tile = osb.tile([128, D], F32, tag="o")
                        o_ps = [ops.tile([128, 512], F32, tag="ops", name=f"ops{qm}_{i}")
                                for i in range(2)]
                        for hh in range(32):
                            for oc in range(2):
                                nc.tensor.matmul(
                                    o_ps[oc][:],
                                    g_sb[hh][:, qm * 128:(qm + 1) * 128],
                                    wo_sb[hh][:, oc * 512:(oc + 1) * 512],
                                    start=(hh == 0), stop=(hh == 31),
                                )
                        for oc in range(2):
                            sl = slice(oc * 512, (oc + 1) * 512)
                            nc.vector.tensor_add(o_tile[:, sl], o_ps[oc][:], x2_sb[qm][:, sl])
                        nc.sync.dma_start(out[qm * 128:(qm + 1) * 128, :], o_tile[:])
                woA_cm.__exit__(None, None, None)
            x2p_cm.__exit__(None, None, None)

    nc.compile()
    return nc


_CACHE = {}


def _get(name, builder):
    if name not in _CACHE:
        _CACHE[name] = builder()
    return _CACHE[name]


def _prep_a_inputs(inputs, xf):
    # fold ln1 gamma/beta into the qkv projection: ln1(x)@W + b =
    # xn@(diag(g)W) + (beta@W + b)
    w = np.asarray(inputs["w_attn"], np.float32)
    g = np.asarray(inputs["ln1_w"], np.float32)
    beta = np.asarray(inputs["ln1_b"], np.float32)
    wattn_bf = (g[:, None] * w).astype(BF)
    battn = np.asarray(inputs["b_attn"], np.float32) + beta @ w
    return [
        dict(x=xf[c * RC:(c + 1) * RC], wattn=wattn_bf, battn=battn)
        for c in range(NCORE)
    ]


def _diagmask():
    """Static triangular masks for slots 0..3: keep where q >= p + 128*s.
    [4*128, 2*RC] bf16, the [128, RC] pattern duplicated for both packed heads."""
    p = np.arange(128)
    q = np.arange(RC)
    dm = np.empty((NDIAG * 128, 2 * RC), dtype=BF)
    for s in range(NDIAG):
        m = (q[None, :] >= (p[:, None] + 128 * s)).astype(BF)
        dm[s * 128:(s + 1) * 128, 0:RC] = m
        dm[s * 128:(s + 1) * 128, RC:2 * RC] = m
    return dm


def _prep_b_inputs(inputs, xf, qkvT):
    """Host reassembly: full K^T/V per batch, per-core slot permutation."""
    kT_b = [np.concatenate([qkvT[4 * b + r][D:2 * D] for r in range(4)], axis=1)
            for b in range(B)]                            # [1024, 2048] bf16
    vT_b = [np.concatenate([qkvT[4 * b + r][2 * D:3 * D] for r in range(4)], axis=1)
            for b in range(B)]
    vaug_b = []
    for b in range(B):
        vn = np.ascontiguousarray(vT_b[b].T)              # [2048, 1024]
        va = np.empty((T, H, DH + 1), dtype=BF)
        va[:, :, :DH] = vn.reshape(T, H, DH)
        va[:, :, DH] = np.ones((), dtype=BF)
        vaug_b.append(va.reshape(T, VA))

    dm = _diagmask()
    # fold ln2 gamma/beta into the fc projection (same identity as ln1)
    wf = np.asarray(inputs["w_fc"], np.float32)
    g2 = np.asarray(inputs["ln2_w"], np.float32)
    beta2 = np.asarray(inputs["ln2_b"], np.float32)
    wfc_bf = (g2[:, None] * wf).astype(BF)
    bfc = np.asarray(inputs["b_fc"], np.float32) + beta2 @ wf
    wproj_bf = np.asarray(inputs["w_proj"], np.float32).astype(BF)
    bproj = np.asarray(inputs["b_proj"], np.float32)
    wout_bf = np.asarray(inputs["w_out"], np.float32).astype(BF)
    bout = np.asarray(inputs["b_out"], np.float32)
    in_maps = []
    for c in range(NCORE):
        b = c // 4
        qb128 = (c % 4) * 4                               # first diagonal key tile
        order = _slot_order(qb128)
        # permute key tiles into slot order (columns of kT, rows of vaug)
        kTp = np.concatenate(
            [kT_b[b][:, k * 128:(k + 1) * 128] for k in order], axis=1)
        vap = np.concatenate(
            [vaug_b[b][k * 128:(k + 1) * 128] for k in order], axis=0)
        # exp bias for interior slots: 0 if fully visible, -1e9 if fully masked
        bias = np.zeros((128, NKT - NDIAG), np.float32)
        for j, k in enumerate(order[NDIAG:]):
            if k > qb128:                                 # above the diagonal band
                bias[:, j] = NEG
        in_maps.append(dict(
            x=xf[c * RC:(c + 1) * RC],
            qT=np.ascontiguousarray(qkvT[c][0:D]),
            kT=np.ascontiguousarray(kTp),
            vaug=np.ascontiguousarray(vap),
            biast=bias,
            diagmask=dm,
            wproj=wproj_bf, bproj=bproj, wfc=wfc_bf, bfc=bfc,
            wout=wout_bf, bout=bout,
        ))
    return in_maps


def kernel(x, ln1_w, ln1_b, ln2_w, ln2_b, w_attn, b_attn, w_proj, b_proj,
           w_fc, b_fc, w_out, b_out):
    inputs = dict(x=x, ln1_w=ln1_w, ln1_b=ln1_b, ln2_w=ln2_w, ln2_b=ln2_b,
                  w_attn=w_attn, b_attn=b_attn, w_proj=w_proj, b_proj=b_proj,
                  w_fc=w_fc, b_fc=b_fc, w_out=w_out, b_out=b_out)
    xf = np.ascontiguousarray(np.asarray(x, np.float32).reshape(R, D))
    cores = list(range(NCORE))

    nc_a = _get("a", _build_a)
    res_a = run_bass_kernel_spmd(nc_a, _prep_a_inputs(inputs, xf), cores).results
    qkvT = [np.asarray(res_a[c]["qkvT"]) for c in cores]  # [3072, 512] bf16

    nc_b = _get("b", _build_b)
    in_maps_b = _prep_b_inputs(inputs, xf, qkvT)
    res_b = run_bass_kernel_spmd(nc_b, in_maps_b, cores).results
    out = np.concatenate([np.asarray(res_b[c]["out"], np.float32) for c in cores], axis=0)
    return out.reshape(B, T, D)


# revision 30
# speedup vs baseline: 1.0132x; 1.0132x over previous
"""Trainium2 Bass kernel for a GPT-style transformer block.

Shapes (hardcoded): x [2, 2048, 1024], n_head=16, causal attention + GELU MLP.
Strategy: row-sharding (4096 rows -> 512 rows/core on 8 cores).
  Launch A: per-core LN1 + qkv projection for own rows -> qkvT [3072, 512] bf16.
  Host:     reassemble full K^T / V per batch; per-core SLOT PERMUTATION of the
            16 key tiles so the 4 diagonal tiles sit at fixed slots 0-3 (their
            triangular masks are then core-independent constants), remaining
            slots carry a per-core {0,-1e9} exp bias that zeroes fully-masked
            tiles for free inside the activation.
  Launch B: per-core attention over own 512 query rows, proj, residual, LN2,
            FFN, residual -> out rows [512, 1024].
All matmuls bf16 with fp32 PSUM accumulation; residual stream / LN / softmax
sums fp32. Attention computes scores for head pairs row-packed on the PE
(K=64 halves at tile_position (0,0)/(64,0)) into one [128, 1024] PSUM span so
exp runs on big tiles (scalar ACTIVATE cost is (N+352)/1.2ns, N = free size).
"""

import sys

if "/opt/trn_rl_repo" not in sys.path:
    sys.path.insert(0, "/opt/trn_rl_repo")

import numpy as np
import ml_dtypes

import concourse.bacc as bacc
import concourse.tile as tile
from concourse import masks
from concourse import mybir
from concourse.bass_utils import run_bass_kernel_spmd

F32 = mybir.dt.float32
BF16 = mybir.dt.bfloat16
AF = mybir.ActivationFunctionType
ALU = mybir.AluOpType
BF = ml_dtypes.bfloat16

B, T, D = 2, 2048, 1024
H, DH = 16, 64
R = B * T          # 4096 flattened rows
NCORE = 8
RC = R // NCORE    # 512 rows per core
EPS = 1e-5
D3 = 3 * D         # 3072
DF = 4 * D         # 4096
VA = H * (DH + 1)  # 1040: V augmented with a ones column per head
NKT = T // 128     # 16 key tiles
NDIAG = 4          # diagonal key tiles per core (512 q rows / 128)
NEG = -1.0e9


def _slot_order(qb128):
    """Key-tile processing order for a core whose queries start at 128*qb128.
    Slots 0..3 = the diagonal tiles; slots 4..15 = the rest ascending."""
    diag = list(range(qb128, qb128 + NDIAG))
    rest = [k for k in range(NKT) if k not in diag]
    return diag + rest


def _layer_norm_tiles(nc, sb, x_tile, out_dtype):
    """Plain LN (no scale/shift: gamma/beta are folded into the downstream
    weights host-side) over free axis of x_tile [128, D] fp32 -> [128, D]."""
    stats = sb.tile([128, 2, 6], F32, tag="ln_stats")
    xg = x_tile[:].rearrange("p (s d) -> p s d", s=2)
    for s in range(2):
        nc.vector.bn_stats(stats[:, s, :], xg[:, s, :])
    mv = sb.tile([128, 2], F32, tag="ln_mv")
    nc.vector.bn_aggr(mv[:], stats[:])
    rstd = sb.tile([128, 1], F32, tag="ln_rstd")
    eps_sb = sb.tile([128, 1], F32, tag="ln_eps")
    nc.vector.memset(eps_sb[:], EPS)
    nc.scalar.activation(rstd[:], mv[:, 1:2], AF.Sqrt, bias=eps_sb[:], scale=1.0)
    nc.vector.reciprocal(rstd[:], rstd[:])
    out = sb.tile([128, D], out_dtype, tag="ln_out")
    nc.vector.tensor_scalar(
        out[:], x_tile[:], mv[:, 0:1], rstd[:], ALU.subtract, ALU.mult,
    )
    return out


def _pe_transpose(nc, tps, dst_tiles, src_tile, t, ident):
    """Transpose src [128, D] bf16 into dst_tiles[j][:, t*128:(t+1)*128]
    via the PE transpose path + copy out of PSUM (split DVE/gpsimd)."""
    for j in range(8):
        tp = tps.tile([128, 128], BF16, tag="tp")
        nc.tensor.transpose(tp[:], src_tile[:, j * 128:(j + 1) * 128], ident[:])
        nc.vector.tensor_copy(dst_tiles[j][:, t * 128:(t + 1) * 128], tp[:])


def _build_a():
    nc = bacc.Bacc("TRN2", target_bir_lowering=False, debug=False, num_devices=NCORE)
    x = nc.dram_tensor("x", [RC, D], F32, kind="ExternalInput")
    wattn = nc.dram_tensor("wattn", [D, D3], BF16, kind="ExternalInput")
    battn = nc.dram_tensor("battn", [D3], F32, kind="ExternalInput")
    qkvT = nc.dram_tensor("qkvT", [D3, RC], BF16, kind="ExternalOutput")

    with tile.TileContext(nc) as tc:
        with (
            tc.tile_pool(name="const", bufs=1) as const,
            tc.tile_pool(name="w", bufs=1) as wpool,
            tc.tile_pool(name="sb", bufs=2) as sb,
            tc.tile_pool(name="lt", bufs=1) as lt,
            tc.tile_pool(name="ps", bufs=4, space="PSUM") as ps,
            tc.tile_pool(name="tps", bufs=3, space="PSUM") as tps,
            tc.tile_pool(name="outp", bufs=3) as outp,
        ):
            # x first (critical path: LN1 -> transpose -> matmul); weights
            # staggered on sync/scalar rings behind it.
            x_sb = [sb.tile([128, D], F32, tag=f"x{t}", name=f"x{t}") for t in range(4)]
            for t in range(4):
                nc.gpsimd.dma_start(x_sb[t][:], x[t * 128:(t + 1) * 128, :])

            ident = const.tile([128, 128], BF16, tag="ident")
            masks.make_identity(nc, ident[:])
            battn_all = const.tile([128, D3 // 128], F32, tag="battn")
            nc.gpsimd.dma_start(battn_all[:], battn.ap().rearrange("(m p) -> p m", p=128))

            w_sb = [wpool.tile([128, D3], BF16, tag=f"w{k}", name=f"w{k}") for k in range(8)]
            for k in range(8):
                eng = nc.sync if k % 2 == 0 else nc.gpsimd
                eng.dma_start(w_sb[k][:], wattn[k * 128:(k + 1) * 128, :])

            ln1T = [lt.tile([128, RC], BF16, tag=f"ln1T{j}", name=f"ln1T{j}") for j in range(8)]
            for t in range(4):
                ln1n = _layer_norm_tiles(nc, sb, x_sb[t], BF16)
                _pe_transpose(nc, tps, ln1T, ln1n, t, ident)

            for m in range(D3 // 128):  # 24 output col-tiles
                psum = ps.tile([128, RC], F32, tag="mm")
                for k in range(8):
                    nc.tensor.matmul(
                        psum[:], w_sb[k][:, m * 128:(m + 1) * 128], ln1T[k][:],
                        start=(k == 0), stop=(k == 7),
                    )
                o_sb = outp.tile([128, RC], BF16, tag="o")
                nc.scalar.activation(
                    o_sb[:], psum[:], AF.Identity,
                    bias=battn_all[:, m:m + 1], scale=1.0,
                )
                nc.sync.dma_start(qkvT[m * 128:(m + 1) * 128, :], o_sb[:])

    nc.compile()
    return nc


def _build_b():
    nc = bacc.Bacc("TRN2", target_bir_lowering=False, debug=False, num_devices=NCORE)
    x = nc.dram_tensor("x", [RC, D], F32, kind="ExternalInput")
    qT = nc.dram_tensor("qT", [D, RC], BF16, kind="ExternalInput")
    kT = nc.dram_tensor("kT", [D, T], BF16, kind="ExternalInput")          # slot-permuted
    vaug = nc.dram_tensor("vaug", [T, VA], BF16, kind="ExternalInput")     # slot-permuted
    biast = nc.dram_tensor("biast", [128, NKT - NDIAG], F32, kind="ExternalInput")
    diagmask = nc.dram_tensor("diagmask", [NDIAG * 128, 2 * RC], BF16, kind="ExternalInput")
    wproj = nc.dram_tensor("wproj", [D, D], BF16, kind="ExternalInput")
    bproj = nc.dram_tensor("bproj", [D], F32, kind="ExternalInput")
    wfc = nc.dram_tensor("wfc", [D, DF], BF16, kind="ExternalInput")
    bfc = nc.dram_tensor("bfc", [DF], F32, kind="ExternalInput")
    wout = nc.dram_tensor("wout", [DF, D], BF16, kind="ExternalInput")
    bout = nc.dram_tensor("bout", [D], F32, kind="ExternalInput")
    out = nc.dram_tensor("out", [RC, D], F32, kind="ExternalOutput")

    with tile.TileContext(nc) as tc:
        with (
            tc.tile_pool(name="const", bufs=1) as const,
            tc.tile_pool(name="att", bufs=1) as attp,
            tc.tile_pool(name="sb", bufs=2) as sb,
        ):
            attT = [attp.tile([128, RC], BF16, tag=f"attT{i}", name=f"attT{i}") for i in range(8)]

            # ---------------- attention ----------------
            with (
                tc.tile_pool(name="kv", bufs=1) as kv,
                tc.tile_pool(name="exps", bufs=20) as exps,
                tc.tile_pool(name="aps", bufs=3, space="PSUM") as aps,
                tc.tile_pool(name="avps", bufs=2, space="PSUM") as avps,
                tc.tile_pool(name="asb", bufs=4) as asb,
            ):
                # critical-path DMAs first: qT/kT per head pair on sync,
                # vaug slots + tiny mask/bias tables early on scalar/gpsimd.
                qT_sb = [kv.tile([128, RC], BF16, tag=f"qT{i}", name=f"qTs{i}") for i in range(8)]
                kT_sb = [kv.tile([128, T], BF16, tag=f"kT{i}", name=f"kTs{i}") for i in range(8)]
                v_sb = [kv.tile([128, VA], BF16, tag=f"v{i}", name=f"vs{i}") for i in range(NKT)]
                dm_sb = [kv.tile([128, 2 * RC], BF16, tag=f"dm{i}", name=f"dms{i}")
                         for i in range(NDIAG)]
                biast_sb = kv.tile([128, NKT - NDIAG], F32, tag="biast")
                nc.gpsimd.dma_start(biast_sb[:], biast[:, :])
                for i in range(NDIAG):
                    nc.gpsimd.dma_start(dm_sb[i][:], diagmask[i * 128:(i + 1) * 128, :])
                for i in range(8):
                    nc.sync.dma_start(qT_sb[i][:], qT[i * 128:(i + 1) * 128, :])
                    nc.sync.dma_start(kT_sb[i][:], kT[i * 128:(i + 1) * 128, :])
                for i in range(NKT):
                    nc.gpsimd.dma_start(v_sb[i][:], vaug[i * 128:(i + 1) * 128, :])

                # weights / residual stream loads overlap the attention phase
                ident = const.tile([128, 128], BF16, tag="ident")
                masks.make_identity(nc, ident[:])
                bproj_bc = const.tile([128, D], F32, tag="bproj")
                nc.gpsimd.dma_start(bproj_bc[:], bproj.ap().partition_broadcast(128))
                bout_bc = const.tile([128, D], F32, tag="bout")
                nc.gpsimd.dma_start(bout_bc[:], bout.ap().partition_broadcast(128))
                bfc_all = const.tile([128, DF // 128], F32, tag="bfc")
                nc.gpsimd.dma_start(bfc_all[:], bfc.ap().rearrange("(m p) -> p m", p=128))

                for hp in range(H // 2):  # head pairs (2hp, 2hp+1)
                    e_tiles = []
                    for s in range(NKT):
                        s_ps = aps.tile([128, 2 * RC], F32, tag="s")
                        nc.tensor.matmul(
                            s_ps[:, 0:RC],
                            kT_sb[hp][0:64, s * 128:(s + 1) * 128],
                            qT_sb[hp][0:64, :],
                            start=True, stop=True, tile_position=(0, 0),
                        )
                        nc.tensor.matmul(
                            s_ps[:, RC:2 * RC],
                            kT_sb[hp][64:128, s * 128:(s + 1) * 128],
                            qT_sb[hp][64:128, :],
                            start=True, stop=True, tile_position=(64, 0),
                        )
                        e_sb = exps.tile([128, 2 * RC], BF16, tag="e")
                        if s < NDIAG:
                            # diagonal tile: plain exp then static triangular mask
                            nc.scalar.activation(e_sb[:], s_ps[:], AF.Exp,
                                                 bias=0.0, scale=0.125)
                            nc.vector.tensor_mul(e_sb[:], e_sb[:], dm_sb[s][:])
                        else:
                            # interior tile: bias is 0 (fully visible) or -1e9
                            # (fully masked -> exp gives exact 0), per-core data
                            nc.scalar.activation(e_sb[:], s_ps[:], AF.Exp,
                                                 bias=biast_sb[:, s - NDIAG:s - NDIAG + 1],
                                                 scale=0.125)
                        e_tiles.append(e_sb)
                    psums = asb.tile([1, 2 * RC], F32, tag="psums")
                    for half in range(2):
                        h = 2 * hp + half
                        hl = half * 64
                        av_ps = avps.tile([DH + 1, RC], F32, tag="av")
                        for s in range(NKT):
                            nc.tensor.matmul(
                                av_ps[:], v_sb[s][:, h * 65:(h + 1) * 65],
                                e_tiles[s][:, half * RC:(half + 1) * RC],
                                start=(s == 0), stop=(s == NKT - 1),
                            )
                        nc.vector.tensor_copy(
                            psums[0:1, half * RC:(half + 1) * RC],
                            av_ps[DH:DH + 1, :])
                        nc.vector.tensor_copy(attT[hp][hl:hl + 64, :], av_ps[0:DH, :])
                    # normalize this pair now; overlaps next pair's QK/exp
                    recips = asb.tile([1, 2 * RC], F32, tag="recips")
                    nc.vector.reciprocal_approx_fast(recips[:], psums[:])
                    r16 = asb.tile([1, 2 * RC], BF16, tag="r16")
                    with nc.allow_low_precision(reason="softmax recip in bf16"):
                        nc.vector.tensor_copy(r16[:], recips[:])
                    rb_sb = asb.tile([128, 2 * RC], BF16, tag="rb")
                    nc.gpsimd.partition_broadcast(rb_sb[:], r16[:])
                    nc.vector.tensor_mul(
                        attT[hp][0:64, :], attT[hp][0:64, :], rb_sb[0:64, 0:RC],
                    )
                    nc.vector.tensor_mul(
                        attT[hp][64:128, :], attT[hp][64:128, :],
                        rb_sb[64:128, RC:2 * RC],
                    )

            # ---------------- proj + residual + LN2 ----------------
            x2p_cm = tc.tile_pool(name="x2p", bufs=1)
            x2p = x2p_cm.__enter__()
            x_sb = [x2p.tile([128, D], F32, tag=f"x{t}", name=f"x{t}") for t in range(4)]
            for t in range(4):
                nc.gpsimd.dma_start(x_sb[t][:], x[t * 128:(t + 1) * 128, :])
                # fold the proj bias into the residual ahead of time (gpsimd,
                # off the critical chain): x_sb becomes x + bproj
                nc.gpsimd.tensor_add(x_sb[t][:], x_sb[t][:], bproj_bc[:])
            ln2T = [x2p.tile([128, RC], BF16, tag=f"ln2T{j}", name=f"ln2T{j}") for j in range(8)]
            x2_sb = [x2p.tile([128, D], F32, tag=f"x2{t}", name=f"x2{t}") for t in range(4)]
            with (
                tc.tile_pool(name="wpj", bufs=1) as wpj,
                tc.tile_pool(name="pps", bufs=3, space="PSUM") as pps,
                tc.tile_pool(name="tps", bufs=3, space="PSUM") as tps,
            ):
                wp_sb = [wpj.tile([128, D], BF16, tag=f"wp{i}", name=f"wp{i}") for i in range(8)]
                for i in range(8):
                    nc.sync.dma_start(wp_sb[i][:], wproj[i * 128:(i + 1) * 128, :])
                for qm in range(4):
                    for oc in range(2):
                        y_ps = pps.tile([128, 512], F32, tag="y")
                        for cc in range(8):
                            nc.tensor.matmul(
                                y_ps[:],
                                attT[cc][:, qm * 128:(qm + 1) * 128],
                                wp_sb[cc][:, oc * 512:(oc + 1) * 512],
                                start=(cc == 0), stop=(cc == 7),
                            )
                        sl = slice(oc * 512, (oc + 1) * 512)
                        nc.vector.tensor_add(x2_sb[qm][:, sl], y_ps[:], x_sb[qm][:, sl])
                    ln2n = _layer_norm_tiles(nc, sb, x2_sb[qm], BF16)
                    _pe_transpose(nc, tps, ln2T, ln2n, qm, ident)
                    # pre-bias the FFN residual in place (WAR on LN2 reads is
                    # tracked by the tile framework)
                    nc.gpsimd.tensor_add(x2_sb[qm][:], x2_sb[qm][:], bout_bc[:])

            # ---------------- FFN ----------------
            with tc.tile_pool(name="g", bufs=1) as gp:
                g_sb = [gp.tile([128, RC], BF16, tag=f"g{i}", name=f"g{i}") for i in range(32)]
                woA_cm = tc.tile_pool(name="woA", bufs=1)
                woA = woA_cm.__enter__()
                # first half of w_out preloads while wfc is still resident
                wo_sb = [woA.tile([128, D], BF16, tag=f"wo{i}", name=f"wos{i}")
                         for i in range(16)]
                for i in range(16):
                    eng = nc.sync if i % 2 == 0 else nc.gpsimd
                    eng.dma_start(wo_sb[i][:], wout[i * 128:(i + 1) * 128, :])
                with (
                    tc.tile_pool(name="wf", bufs=1) as wf,
                    tc.tile_pool(name="fps", bufs=4, space="PSUM") as fps,
                ):
                    wf_sb = [wf.tile([128, DF], BF16, tag=f"wf{i}", name=f"wfs{i}") for i in range(8)]
                    for i in range(8):
                        eng = nc.sync if i % 2 == 0 else nc.gpsimd
                        eng.dma_start(wf_sb[i][:], wfc[i * 128:(i + 1) * 128, :])
                    for hm in range(32):
                        h_ps = fps.tile([128, RC], F32, tag="h")
                        # two sequential complete accumulation groups per psum
                        # (halves of the query range): the first only needs the
                        # first two LN2 transposes, overlapping the proj tail
                        for qh in range(2):
                            sl = slice(qh * 256, (qh + 1) * 256)
                            for k in range(8):
                                nc.tensor.matmul(
                                    h_ps[:, sl], wf_sb[k][:, hm * 128:(hm + 1) * 128],
                                    ln2T[k][:, sl], start=(k == 0), stop=(k == 7),
                                )
                        nc.scalar.activation(
                            g_sb[hm][:], h_ps[:], AF.Gelu,
                            bias=bfc_all[:, hm:hm + 1], scale=1.0,
                        )

                with (
                    tc.tile_pool(name="woB", bufs=1) as woB,
                    tc.tile_pool(name="ops", bufs=4, space="PSUM") as ops,
                    tc.tile_pool(name="osb", bufs=3) as osb,
                ):
                    wo_sb += [woB.tile([128, D], BF16, tag=f"wo{i}", name=f"wosb{i}")
                              for i in range(16, 32)]
                    for i in range(16, 32):
                        eng = nc.sync if i % 2 == 0 else nc.gpsimd
                        eng.dma_start(wo_sb[i][:], wout[i * 128:(i + 1) * 128, :])
                    for qm in range(4):
                        o_tile = osb.tile([128, D], F32, tag="o")
                        o_ps = [ops.tile([128, 512], F32, tag="ops", name=f"ops{qm}_{i}")
                                for i in range(2)]
                        for hh in range(32):
                            for oc in range(2):
                                nc.tensor.matmul(
                                    o_ps[oc][:],
                                    g_sb[hh][:, qm * 128:(qm + 1) * 128],
                                    wo_sb[hh][:, oc * 512:(oc + 1) * 512],
                                    start=(hh == 0), stop=(hh == 31),
                                )
                        for oc in range(2):
                            sl = slice(oc * 512, (oc + 1) * 512)
                            nc.vector.tensor_add(o_tile[:, sl], o_ps[oc][:], x2_sb[qm][:, sl])
                        nc.sync.dma_start(out[qm * 128:(qm + 1) * 128, :], o_tile[:])
                woA_cm.__exit__(None, None, None)
            x2p_cm.__exit__(None, None, None)

    nc.compile()
    return nc


_CACHE = {}


def _get(name, builder):
    if name not in _CACHE:
        _CACHE[name] = builder()
    return _CACHE[name]


def _prep_a_inputs(inputs, xf):
    # fold ln1 gamma/beta into the qkv projection: ln1(x)@W + b =
    # xn@(diag(g)W) + (beta@W + b)
    w = np.asarray(inputs["w_attn"], np.float32)
    g = np.asarray(inputs["ln1_w"], np.float32)
    beta = np.asarray(inputs["ln1_b"], np.float32)
    wattn_bf = (g[:, None] * w).astype(BF)
    battn = np.asarray(inputs["b_attn"], np.float32) + beta @ w
    return [
        dict(x=xf[c * RC:(c + 1) * RC], wattn=wattn_bf, battn=battn)
        for c in range(NCORE)
    ]


def _diagmask():
    """Static triangular masks for slots 0..3: keep where q >= p + 128*s.
    [4*128, 2*RC] bf16, the [128, RC] pattern duplicated for both packed heads."""
    p = np.arange(128)
    q = np.arange(RC)
    dm = np.empty((NDIAG * 128, 2 * RC), dtype=BF)
    for s in range(NDIAG):
        m = (q[None, :] >= (p[:, None] + 128 * s)).astype(BF)
        dm[s * 128:(s + 1) * 128, 0:RC] = m
        dm[s * 128:(s + 1) * 128, RC:2 * RC] = m
    return dm


def _prep_b_inputs(inputs, xf, qkvT):
    """Host reassembly: full K^T/V per batch, per-core slot permutation."""
    kT_b = [np.concatenate([qkvT[4 * b + r][D:2 * D] for r in range(4)], axis=1)
            for b in range(B)]                            # [1024, 2048] bf16
    vT_b = [np.concatenate([qkvT[4 * b + r][2 * D:3 * D] for r in range(4)], axis=1)
            for b in range(B)]
    vaug_b = []
    for b in range(B):
        vn = np.ascontiguousarray(vT_b[b].T)              # [2048, 1024]
        va = np.empty((T, H, DH + 1), dtype=BF)
        va[:, :, :DH] = vn.reshape(T, H, DH)
        va[:, :, DH] = np.ones((), dtype=BF)
        vaug_b.append(va.reshape(T, VA))

    dm = _diagmask()
    # fold ln2 gamma/beta into the fc projection (same identity as ln1)
    wf = np.asarray(inputs["w_fc"], np.float32)
    g2 = np.asarray(inputs["ln2_w"], np.float32)
    beta2 = np.asarray(inputs["ln2_b"], np.float32)
    wfc_bf = (g2[:, None] * wf).astype(BF)
    bfc = np.asarray(inputs["b_fc"], np.float32) + beta2 @ wf
    wproj_bf = np.asarray(inputs["w_proj"], np.float32).astype(BF)
    bproj = np.asarray(inputs["b_proj"], np.float32)
    wout_bf = np.asarray(inputs["w_out"], np.float32).astype(BF)
    bout = np.asarray(inputs["b_out"], np.float32)
    in_maps = []
    for c in range(NCORE):
        b = c // 4
        qb128 = (c % 4) * 4                               # first diagonal key tile
        order = _slot_order(qb128)
        # permute key tiles into slot order (columns of kT, rows of vaug)
        kTp = np.concatenate(
            [kT_b[b][:, k * 128:(k + 1) * 128] for k in order], axis=1)
        vap = np.concatenate(
            [vaug_b[b][k * 128:(k + 1) * 128] for k in order], axis=0)
        # exp bias for interior slots: 0 if fully visible, -1e9 if fully masked
        bias = np.zeros((128, NKT - NDIAG), np.float32)
        for j, k in enumerate(order[NDIAG:]):
            if k > qb128:                                 # above the diagonal band
                bias[:, j] = NEG
        in_maps.append(dict(
            x=xf[c * RC:(c + 1) * RC],
            qT=np.ascontiguousarray(qkvT[c][0:D]),
            kT=np.ascontiguousarray(kTp),
            vaug=np.ascontiguousarray(vap),
            biast=bias,
            diagmask=dm,
            wproj=wproj_bf, bproj=bproj, wfc=wfc_bf, bfc=bfc,
            wout=wout_bf, bout=bout,
        ))
    return in_maps


def kernel(x, ln1_w, ln1_b, ln2_w, ln2_b, w_attn, b_attn, w_proj, b_proj,
           w_fc, b_fc, w_out, b_out):
    inputs = dict(x=x, ln1_w=ln1_w, ln1_b=ln1_b, ln2_w=ln2_w, ln2_b=ln2_b,
                  w_attn=w_attn, b_attn=b_attn, w_proj=w_proj, b_proj=b_proj,
                  w_fc=w_fc, b_fc=b_fc, w_out=w_out, b_out=b_out)
    xf = np.ascontiguousarray(np.asarray(x, np.float32).reshape(R, D))
    cores = list(range(NCORE))

    nc_a = _get("a", _build_a)
    res_a = run_bass_kernel_spmd(nc_a, _prep_a_inputs(inputs, xf), cores).results
    qkvT = [np.asarray(res_a[c]["qkvT"]) for c in cores]  # [3072, 512] bf16

    nc_b = _get("b", _build_b)
    in_maps_b = _prep_b_inputs(inputs, xf, qkvT)
    res_b = run_bass_kernel_spmd(nc_b, in_maps_b, cores).results
    out = np.concatenate([np.asarray(res_b[c]["out"], np.float32) for c in cores], axis=0)
    return out.reshape(B, T, D)


# revision 31
# speedup vs baseline: 1.0243x; 1.0109x over previous
"""Trainium2 Bass kernel for a GPT-style transformer block.

Shapes (hardcoded): x [2, 2048, 1024], n_head=16, causal attention + GELU MLP.
Strategy: row-sharding (4096 rows -> 512 rows/core on 8 cores).
  Launch A: per-core LN1 + qkv projection for own rows -> qkvT [3072, 512] bf16.
  Host:     reassemble full K^T / V per batch; per-core SLOT PERMUTATION of the
            16 key tiles so the 4 diagonal tiles sit at fixed slots 0-3 (their
            triangular masks are then core-independent constants), remaining
            slots carry a per-core {0,-1e9} exp bias that zeroes fully-masked
            tiles for free inside the activation.
  Launch B: per-core attention over own 512 query rows, proj, residual, LN2,
            FFN, residual -> out rows [512, 1024].
All matmuls bf16 with fp32 PSUM accumulation; residual stream / LN / softmax
sums fp32. Attention computes scores for head pairs row-packed on the PE
(K=64 halves at tile_position (0,0)/(64,0)) into one [128, 1024] PSUM span so
exp runs on big tiles (scalar ACTIVATE cost is (N+352)/1.2ns, N = free size).
"""

import sys

if "/opt/trn_rl_repo" not in sys.path:
    sys.path.insert(0, "/opt/trn_rl_repo")

import numpy as np
import ml_dtypes

import concourse.bacc as bacc
import concourse.tile as tile
from concourse import masks
from concourse import mybir
from concourse.bass_utils import run_bass_kernel_spmd

F32 = mybir.dt.float32
BF16 = mybir.dt.bfloat16
AF = mybir.ActivationFunctionType
ALU = mybir.AluOpType
BF = ml_dtypes.bfloat16

B, T, D = 2, 2048, 1024
H, DH = 16, 64
R = B * T          # 4096 flattened rows
NCORE = 8
RC = R // NCORE    # 512 rows per core
EPS = 1e-5
D3 = 3 * D         # 3072
DF = 4 * D         # 4096
VA = H * (DH + 1)  # 1040: V augmented with a ones column per head
NKT = T // 128     # 16 key tiles
NDIAG = 4          # diagonal key tiles per core (512 q rows / 128)
NEG = -1.0e9


def _slot_order(qb128):
    """Key-tile processing order for a core whose queries start at 128*qb128.
    Slots 0..3 = the diagonal tiles; slots 4..15 = the rest ascending."""
    diag = list(range(qb128, qb128 + NDIAG))
    rest = [k for k in range(NKT) if k not in diag]
    return diag + rest


def _layer_norm_tiles(nc, sb, x_tile, out_dtype):
    """Plain LN (no scale/shift: gamma/beta are folded into the downstream
    weights host-side) over free axis of x_tile [128, D] fp32 -> [128, D]."""
    stats = sb.tile([128, 2, 6], F32, tag="ln_stats")
    xg = x_tile[:].rearrange("p (s d) -> p s d", s=2)
    for s in range(2):
        nc.vector.bn_stats(stats[:, s, :], xg[:, s, :])
    mv = sb.tile([128, 2], F32, tag="ln_mv")
    nc.vector.bn_aggr(mv[:], stats[:])
    rstd = sb.tile([128, 1], F32, tag="ln_rstd")
    eps_sb = sb.tile([128, 1], F32, tag="ln_eps")
    nc.vector.memset(eps_sb[:], EPS)
    nc.scalar.activation(rstd[:], mv[:, 1:2], AF.Sqrt, bias=eps_sb[:], scale=1.0)
    nc.vector.reciprocal(rstd[:], rstd[:])
    out = sb.tile([128, D], out_dtype, tag="ln_out")
    nc.vector.tensor_scalar(
        out[:], x_tile[:], mv[:, 0:1], rstd[:], ALU.subtract, ALU.mult,
    )
    return out


def _pe_transpose(nc, tps, dst_tiles, src_tile, t, ident):
    """Transpose src [128, D] bf16 into dst_tiles[j][:, t*128:(t+1)*128]
    via the PE transpose path + copy out of PSUM (split DVE/gpsimd)."""
    for j in range(8):
        tp = tps.tile([128, 128], BF16, tag="tp")
        nc.tensor.transpose(tp[:], src_tile[:, j * 128:(j + 1) * 128], ident[:])
        nc.vector.tensor_copy(dst_tiles[j][:, t * 128:(t + 1) * 128], tp[:])


def _build_a():
    nc = bacc.Bacc("TRN2", target_bir_lowering=False, debug=False, num_devices=NCORE)
    x = nc.dram_tensor("x", [RC, D], F32, kind="ExternalInput")
    wattn = nc.dram_tensor("wattn", [D, D3], BF16, kind="ExternalInput")
    battn = nc.dram_tensor("battn", [D3], F32, kind="ExternalInput")
    qkvT = nc.dram_tensor("qkvT", [D3, RC], BF16, kind="ExternalOutput")

    with tile.TileContext(nc) as tc:
        with (
            tc.tile_pool(name="const", bufs=1) as const,
            tc.tile_pool(name="w", bufs=1) as wpool,
            tc.tile_pool(name="sb", bufs=2) as sb,
            tc.tile_pool(name="lt", bufs=1) as lt,
            tc.tile_pool(name="ps", bufs=4, space="PSUM") as ps,
            tc.tile_pool(name="tps", bufs=3, space="PSUM") as tps,
            tc.tile_pool(name="outp", bufs=3) as outp,
        ):
            # x first (critical path: LN1 -> transpose -> matmul); weights
            # staggered on sync/scalar rings behind it.
            x_sb = [sb.tile([128, D], F32, tag=f"x{t}", name=f"x{t}") for t in range(4)]
            for t in range(4):
                nc.gpsimd.dma_start(x_sb[t][:], x[t * 128:(t + 1) * 128, :])

            ident = const.tile([128, 128], BF16, tag="ident")
            masks.make_identity(nc, ident[:])
            battn_all = const.tile([128, D3 // 128], F32, tag="battn")
            nc.gpsimd.dma_start(battn_all[:], battn.ap().rearrange("(m p) -> p m", p=128))

            w_sb = [wpool.tile([128, D3], BF16, tag=f"w{k}", name=f"w{k}") for k in range(8)]
            for k in range(8):
                eng = nc.sync if k % 2 == 0 else nc.gpsimd
                eng.dma_start(w_sb[k][:], wattn[k * 128:(k + 1) * 128, :])

            ln1T = [lt.tile([128, RC], BF16, tag=f"ln1T{j}", name=f"ln1T{j}") for j in range(8)]
            for t in range(4):
                ln1n = _layer_norm_tiles(nc, sb, x_sb[t], BF16)
                _pe_transpose(nc, tps, ln1T, ln1n, t, ident)

            for m in range(D3 // 128):  # 24 output col-tiles
                psum = ps.tile([128, RC], F32, tag="mm")
                for k in range(8):
                    nc.tensor.matmul(
                        psum[:], w_sb[k][:, m * 128:(m + 1) * 128], ln1T[k][:],
                        start=(k == 0), stop=(k == 7),
                    )
                o_sb = outp.tile([128, RC], BF16, tag="o")
                nc.scalar.activation(
                    o_sb[:], psum[:], AF.Identity,
                    bias=battn_all[:, m:m + 1], scale=1.0,
                )
                nc.sync.dma_start(qkvT[m * 128:(m + 1) * 128, :], o_sb[:])

    nc.compile()
    return nc


def _build_b():
    nc = bacc.Bacc("TRN2", target_bir_lowering=False, debug=False, num_devices=NCORE)
    x = nc.dram_tensor("x", [RC, D], F32, kind="ExternalInput")
    qT = nc.dram_tensor("qT", [D, RC], BF16, kind="ExternalInput")
    kT = nc.dram_tensor("kT", [D, T], BF16, kind="ExternalInput")          # slot-permuted
    vaug = nc.dram_tensor("vaug", [T, VA], BF16, kind="ExternalInput")     # slot-permuted
    biast = nc.dram_tensor("biast", [128, NKT - NDIAG], F32, kind="ExternalInput")
    diagmask = nc.dram_tensor("diagmask", [NDIAG * 128, 2 * RC], BF16, kind="ExternalInput")
    wproj = nc.dram_tensor("wproj", [D, D], BF16, kind="ExternalInput")
    bproj = nc.dram_tensor("bproj", [D], F32, kind="ExternalInput")
    wfc = nc.dram_tensor("wfc", [D, DF], BF16, kind="ExternalInput")
    bfc = nc.dram_tensor("bfc", [DF], F32, kind="ExternalInput")
    wout = nc.dram_tensor("wout", [DF, D], BF16, kind="ExternalInput")
    bout = nc.dram_tensor("bout", [D], F32, kind="ExternalInput")
    out = nc.dram_tensor("out", [RC, D], F32, kind="ExternalOutput")

    with tile.TileContext(nc) as tc:
        with (
            tc.tile_pool(name="const", bufs=1) as const,
            tc.tile_pool(name="att", bufs=1) as attp,
            tc.tile_pool(name="sb", bufs=2) as sb,
        ):
            attT = [attp.tile([128, RC], BF16, tag=f"attT{i}", name=f"attT{i}") for i in range(8)]

            # ---------------- attention ----------------
            with (
                tc.tile_pool(name="kv", bufs=1) as kv,
                tc.tile_pool(name="exps", bufs=20) as exps,
                tc.tile_pool(name="aps", bufs=3, space="PSUM") as aps,
                tc.tile_pool(name="avps", bufs=2, space="PSUM") as avps,
                tc.tile_pool(name="asb", bufs=4) as asb,
            ):
                # critical-path DMAs first: qT/kT per head pair on sync,
                # vaug slots + tiny mask/bias tables early on scalar/gpsimd.
                qT_sb = [kv.tile([128, RC], BF16, tag=f"qT{i}", name=f"qTs{i}") for i in range(8)]
                kT_sb = [kv.tile([128, T], BF16, tag=f"kT{i}", name=f"kTs{i}") for i in range(8)]
                v_sb = [kv.tile([128, VA], BF16, tag=f"v{i}", name=f"vs{i}") for i in range(NKT)]
                dm_sb = [kv.tile([128, 2 * RC], BF16, tag=f"dm{i}", name=f"dms{i}")
                         for i in range(NDIAG)]
                biast_sb = kv.tile([128, NKT - NDIAG], F32, tag="biast")
                nc.gpsimd.dma_start(biast_sb[:], biast[:, :])
                for i in range(NDIAG):
                    nc.gpsimd.dma_start(dm_sb[i][:], diagmask[i * 128:(i + 1) * 128, :])
                for i in range(8):
                    nc.sync.dma_start(qT_sb[i][:], qT[i * 128:(i + 1) * 128, :])
                    nc.sync.dma_start(kT_sb[i][:], kT[i * 128:(i + 1) * 128, :])
                for i in range(NKT):
                    nc.gpsimd.dma_start(v_sb[i][:], vaug[i * 128:(i + 1) * 128, :])

                # weights / residual stream loads overlap the attention phase
                ident = const.tile([128, 128], BF16, tag="ident")
                masks.make_identity(nc, ident[:])
                bproj_bc = const.tile([128, D], F32, tag="bproj")
                nc.gpsimd.dma_start(bproj_bc[:], bproj.ap().partition_broadcast(128))
                bout_bc = const.tile([128, D], F32, tag="bout")
                nc.gpsimd.dma_start(bout_bc[:], bout.ap().partition_broadcast(128))
                bfc_all = const.tile([128, DF // 128], F32, tag="bfc")
                nc.gpsimd.dma_start(bfc_all[:], bfc.ap().rearrange("(m p) -> p m", p=128))

                for hp in range(H // 2):  # head pairs (2hp, 2hp+1)
                    e_tiles = []
                    for s in range(NKT):
                        s_ps = aps.tile([128, 2 * RC], F32, tag="s")
                        nc.tensor.matmul(
                            s_ps[:, 0:RC],
                            kT_sb[hp][0:64, s * 128:(s + 1) * 128],
                            qT_sb[hp][0:64, :],
                            start=True, stop=True, tile_position=(0, 0),
                        )
                        nc.tensor.matmul(
                            s_ps[:, RC:2 * RC],
                            kT_sb[hp][64:128, s * 128:(s + 1) * 128],
                            qT_sb[hp][64:128, :],
                            start=True, stop=True, tile_position=(64, 0),
                        )
                        e_sb = exps.tile([128, 2 * RC], BF16, tag="e")
                        if s < NDIAG:
                            # diagonal tile: plain exp then static triangular mask
                            nc.scalar.activation(e_sb[:], s_ps[:], AF.Exp,
                                                 bias=0.0, scale=0.125)
                            nc.vector.tensor_mul(e_sb[:], e_sb[:], dm_sb[s][:])
                        else:
                            # interior tile: bias is 0 (fully visible) or -1e9
                            # (fully masked -> exp gives exact 0), per-core data
                            nc.scalar.activation(e_sb[:], s_ps[:], AF.Exp,
                                                 bias=biast_sb[:, s - NDIAG:s - NDIAG + 1],
                                                 scale=0.125)
                        e_tiles.append(e_sb)
                    psums = asb.tile([1, 2 * RC], F32, tag="psums")
                    for half in range(2):
                        h = 2 * hp + half
                        hl = half * 64
                        av_ps = avps.tile([DH + 1, RC], F32, tag="av")
                        for s in range(NKT):
                            nc.tensor.matmul(
                                av_ps[:], v_sb[s][:, h * 65:(h + 1) * 65],
                                e_tiles[s][:, half * RC:(half + 1) * RC],
                                start=(s == 0), stop=(s == NKT - 1),
                            )
                        nc.vector.tensor_copy(
                            psums[0:1, half * RC:(half + 1) * RC],
                            av_ps[DH:DH + 1, :])
                        nc.vector.tensor_copy(attT[hp][hl:hl + 64, :], av_ps[0:DH, :])
                    # normalize this pair now; overlaps next pair's QK/exp
                    recips = asb.tile([1, 2 * RC], F32, tag="recips")
                    nc.vector.reciprocal_approx_fast(recips[:], psums[:])
                    r16 = asb.tile([1, 2 * RC], BF16, tag="r16")
                    with nc.allow_low_precision(reason="softmax recip in bf16"):
                        nc.vector.tensor_copy(r16[:], recips[:])
                    rb_sb = asb.tile([128, 2 * RC], BF16, tag="rb")
                    nc.gpsimd.partition_broadcast(rb_sb[:], r16[:])
                    nc.vector.tensor_mul(
                        attT[hp][0:64, :], attT[hp][0:64, :], rb_sb[0:64, 0:RC],
                    )
                    nc.vector.tensor_mul(
                        attT[hp][64:128, :], attT[hp][64:128, :],
                        rb_sb[64:128, RC:2 * RC],
                    )

            # ---------------- proj + residual + LN2 ----------------
            x2p_cm = tc.tile_pool(name="x2p", bufs=1)
            x2p = x2p_cm.__enter__()
            x_sb = [x2p.tile([128, D], F32, tag=f"x{t}", name=f"x{t}") for t in range(4)]
            for t in range(4):
                nc.gpsimd.dma_start(x_sb[t][:], x[t * 128:(t + 1) * 128, :])
                # fold the proj bias into the residual ahead of time (gpsimd,
                # off the critical chain): x_sb becomes x + bproj
                nc.gpsimd.tensor_add(x_sb[t][:], x_sb[t][:], bproj_bc[:])
            ln2T = [x2p.tile([128, RC], BF16, tag=f"ln2T{j}", name=f"ln2T{j}") for j in range(8)]
            x2_sb = [x2p.tile([128, D], F32, tag=f"x2{t}", name=f"x2{t}") for t in range(4)]
            with (
                tc.tile_pool(name="wpj", bufs=1) as wpj,
                tc.tile_pool(name="pps", bufs=3, space="PSUM") as pps,
                tc.tile_pool(name="tps", bufs=3, space="PSUM") as tps,
            ):
                wp_sb = [wpj.tile([128, D], BF16, tag=f"wp{i}", name=f"wp{i}") for i in range(8)]
                for i in range(8):
                    nc.sync.dma_start(wp_sb[i][:], wproj[i * 128:(i + 1) * 128, :])
                for qm in range(4):
                    for oc in range(2):
                        y_ps = pps.tile([128, 512], F32, tag="y")
                        for cc in range(8):
                            nc.tensor.matmul(
                                y_ps[:],
                                attT[cc][:, qm * 128:(qm + 1) * 128],
                                wp_sb[cc][:, oc * 512:(oc + 1) * 512],
                                start=(cc == 0), stop=(cc == 7),
                            )
                        sl = slice(oc * 512, (oc + 1) * 512)
                        nc.vector.tensor_add(x2_sb[qm][:, sl], y_ps[:], x_sb[qm][:, sl])
                    ln2n = _layer_norm_tiles(nc, sb, x2_sb[qm], BF16)
                    _pe_transpose(nc, tps, ln2T, ln2n, qm, ident)
                    # pre-bias the FFN residual in place (WAR on LN2 reads is
                    # tracked by the tile framework)
                    nc.gpsimd.tensor_add(x2_sb[qm][:], x2_sb[qm][:], bout_bc[:])

            # ---------------- FFN ----------------
            with tc.tile_pool(name="g", bufs=1) as gp:
                g_sb = [gp.tile([128, RC], BF16, tag=f"g{i}", name=f"g{i}") for i in range(32)]
                woA_cm = tc.tile_pool(name="woA", bufs=1)
                woA = woA_cm.__enter__()
                # first half of w_out preloads while wfc is still resident
                wo_sb = [woA.tile([128, D], BF16, tag=f"wo{i}", name=f"wos{i}")
                         for i in range(16)]
                for i in range(16):
                    eng = nc.sync if i % 2 == 0 else nc.gpsimd
                    eng.dma_start(wo_sb[i][:], wout[i * 128:(i + 1) * 128, :])
                with (
                    tc.tile_pool(name="wf", bufs=1) as wf,
                    tc.tile_pool(name="fps", bufs=4, space="PSUM") as fps,
                ):
                    wf_sb = [wf.tile([128, DF], BF16, tag=f"wf{i}", name=f"wfs{i}") for i in range(8)]
                    for i in range(8):
                        eng = nc.sync if i % 2 == 0 else nc.gpsimd
                        eng.dma_start(wf_sb[i][:], wfc[i * 128:(i + 1) * 128, :])
                    for hm in range(32):
                        h_ps = fps.tile([128, RC], F32, tag="h")
                        for k in range(8):
                            nc.tensor.matmul(
                                h_ps[:], wf_sb[k][:, hm * 128:(hm + 1) * 128],
                                ln2T[k][:], start=(k == 0), stop=(k == 7),
                            )
                        nc.scalar.activation(
                            g_sb[hm][:], h_ps[:], AF.Gelu,
                            bias=bfc_all[:, hm:hm + 1], scale=1.0,
                        )

                with (
                    tc.tile_pool(name="woB", bufs=1) as woB,
                    tc.tile_pool(name="ops", bufs=4, space="PSUM") as ops,
                    tc.tile_pool(name="osb", bufs=3) as osb,
                ):
                    wo_sb += [woB.tile([128, D], BF16, tag=f"wo{i}", name=f"wosb{i}")
                              for i in range(16, 32)]
                    for i in range(16, 32):
                        eng = nc.sync if i % 2 == 0 else nc.gpsimd
                        eng.dma_start(wo_sb[i][:], wout[i * 128:(i + 1) * 128, :])
                    for qm in range(4):
                        o_tile = osb.tile([128, D], F32, tag="o")
                        o_ps = [ops.tile([128, 512], F32, tag="ops", name=f"ops{qm}_{i}")
                                for i in range(2)]
                        for hh in range(32):
                            for oc in range(2):
                                nc.tensor.matmul(
                                    o_ps[oc][:],
                                    g_sb[hh][:, qm * 128:(qm + 1) * 128],
                                    wo_sb[hh][:, oc * 512:(oc + 1) * 512],
                                    start=(hh == 0), stop=(hh == 31),
                                )
                        for oc in range(2):
                            sl = slice(oc * 512, (oc + 1) * 512)
                            nc.vector.tensor_add(o_tile[:, sl], o_ps[oc][:], x2_sb[qm][:, sl])
                        nc.sync.dma_start(out[qm * 128:(qm + 1) * 128, :], o_tile[:])
                woA_cm.__exit__(None, None, None)
            x2p_cm.__exit__(None, None, None)

    nc.compile()
    return nc


_CACHE = {}


def _get(name, builder):
    if name not in _CACHE:
        _CACHE[name] = builder()
    return _CACHE[name]


def _prep_a_inputs(inputs, xf):
    # fold ln1 gamma/beta into the qkv projection: ln1(x)@W + b =
    # xn@(diag(g)W) + (beta@W + b)
    w = np.asarray(inputs["w_attn"], np.float32)
    g = np.asarray(inputs["ln1_w"], np.float32)
    beta = np.asarray(inputs["ln1_b"], np.float32)
    wattn_bf = (g[:, None] * w).astype(BF)
    battn = np.asarray(inputs["b_attn"], np.float32) + beta @ w
    return [
        dict(x=xf[c * RC:(c + 1) * RC], wattn=wattn_bf, battn=battn)
        for c in range(NCORE)
    ]


def _diagmask():
    """Static triangular masks for slots 0..3: keep where q >= p + 128*s.
    [4*128, 2*RC] bf16, the [128, RC] pattern duplicated for both packed heads."""
    p = np.arange(128)
    q = np.arange(RC)
    dm = np.empty((NDIAG * 128, 2 * RC), dtype=BF)
    for s in range(NDIAG):
        m = (q[None, :] >= (p[:, None] + 128 * s)).astype(BF)
        dm[s * 128:(s + 1) * 128, 0:RC] = m
        dm[s * 128:(s + 1) * 128, RC:2 * RC] = m
    return dm


def _prep_b_inputs(inputs, xf, qkvT):
    """Host reassembly: full K^T/V per batch, per-core slot permutation."""
    kT_b = [np.concatenate([qkvT[4 * b + r][D:2 * D] for r in range(4)], axis=1)
            for b in range(B)]                            # [1024, 2048] bf16
    vT_b = [np.concatenate([qkvT[4 * b + r][2 * D:3 * D] for r in range(4)], axis=1)
            for b in range(B)]
    vaug_b = []
    for b in range(B):
        vn = np.ascontiguousarray(vT_b[b].T)              # [2048, 1024]
        va = np.empty((T, H, DH + 1), dtype=BF)
        va[:, :, :DH] = vn.reshape(T, H, DH)
        va[:, :, DH] = np.ones((), dtype=BF)
        vaug_b.append(va.reshape(T, VA))

    dm = _diagmask()
    # fold ln2 gamma/beta into the fc projection (same identity as ln1)
    wf = np.asarray(inputs["w_fc"], np.float32)
    g2 = np.asarray(inputs["ln2_w"], np.float32)
    beta2 = np.asarray(inputs["ln2_b"], np.float32)
    wfc_bf = (g2[:, None] * wf).astype(BF)
    bfc = np.asarray(inputs["b_fc"], np.float32) + beta2 @ wf
    wproj_bf = np.asarray(inputs["w_proj"], np.float32).astype(BF)
    bproj = np.asarray(inputs["b_proj"], np.float32)
    wout_bf = np.asarray(inputs["w_out"], np.float32).astype(BF)
    bout = np.asarray(inputs["b_out"], np.float32)
    in_maps = []
    for c in range(NCORE):
        b = c // 4
        qb128 = (c % 4) * 4                               # first diagonal key tile
        order = _slot_order(qb128)
        # permute key tiles into slot order (columns of kT, rows of vaug)
        kTp = np.concatenate(
            [kT_b[b][:, k * 128:(k + 1) * 128] for k in order], axis=1)
        vap = np.concatenate(
            [vaug_b[b][k * 128:(k + 1) * 128] for k in order], axis=0)
        # exp bias for interior slots: 0 if fully visible, -1e9 if fully masked
        bias = np.zeros((128, NKT - NDIAG), np.float32)
        for j, k in enumerate(order[NDIAG:]):
            if k > qb128:                                 # above the diagonal band
                bias[:, j] = NEG
        in_maps.append(dict(
            x=xf[c * RC:(c + 1) * RC],
            qT=np.ascontiguousarray(qkvT[c][0:D]),
            kT=np.ascontiguousarray(kTp),
            vaug=np.ascontiguousarray(vap),
            biast=bias,
            diagmask=dm,
            wproj=wproj_bf, bproj=bproj, wfc=wfc_bf, bfc=bfc,
            wout=wout_bf, bout=bout,
        ))
    return in_maps


def kernel(x, ln1_w, ln1_b, ln2_w, ln2_b, w_attn, b_attn, w_proj, b_proj,
           w_fc, b_fc, w_out, b_out):
    inputs = dict(x=x, ln1_w=ln1_w, ln1_b=ln1_b, ln2_w=ln2_w, ln2_b=ln2_b,
                  w_attn=w_attn, b_attn=b_attn, w_proj=w_proj, b_proj=b_proj,
                  w_fc=w_fc, b_fc=b_fc, w_out=w_out, b_out=b_out)
    xf = np.ascontiguousarray(np.asarray(x, np.float32).reshape(R, D))
    cores = list(range(NCORE))

    nc_a = _get("a", _build_a)
    res_a = run_bass_kernel_spmd(nc_a, _prep_a_inputs(inputs, xf), cores).results
    qkvT = [np.asarray(res_a[c]["qkvT"]) for c in cores]  # [3072, 512] bf16

    nc_b = _get("b", _build_b)
    in_maps_b = _prep_b_inputs(inputs, xf, qkvT)
    res_b = run_bass_kernel_spmd(nc_b, in_maps_b, cores).results
    out = np.concatenate([np.asarray(res_b[c]["out"], np.float32) for c in cores], axis=0)
    return out.reshape(B, T, D)


# revision 33
# speedup vs baseline: 1.0276x; 1.0032x over previous
"""Trainium2 Bass kernel for a GPT-style transformer block.

Shapes (hardcoded): x [2, 2048, 1024], n_head=16, causal attention + GELU MLP.
Strategy: row-sharding (4096 rows -> 512 rows/core on 8 cores).
  Launch A: per-core LN1 + qkv projection for own rows -> qkvT [3072, 512] bf16.
  Host:     reassemble full K^T / V per batch; per-core SLOT PERMUTATION of the
            16 key tiles so the 4 diagonal tiles sit at fixed slots 0-3 (their
            triangular masks are then core-independent constants), remaining
            slots carry a per-core {0,-1e9} exp bias that zeroes fully-masked
            tiles for free inside the activation.
  Launch B: per-core attention over own 512 query rows, proj, residual, LN2,
            FFN, residual -> out rows [512, 1024].
All matmuls bf16 with fp32 PSUM accumulation; residual stream / LN / softmax
sums fp32. Attention computes scores for head pairs row-packed on the PE
(K=64 halves at tile_position (0,0)/(64,0)) into one [128, 1024] PSUM span so
exp runs on big tiles (scalar ACTIVATE cost is (N+352)/1.2ns, N = free size).
"""

import sys

if "/opt/trn_rl_repo" not in sys.path:
    sys.path.insert(0, "/opt/trn_rl_repo")

import numpy as np
import ml_dtypes

import concourse.bacc as bacc
import concourse.tile as tile
from concourse import masks
from concourse import mybir
from concourse.bass_utils import run_bass_kernel_spmd

F32 = mybir.dt.float32
BF16 = mybir.dt.bfloat16
AF = mybir.ActivationFunctionType
ALU = mybir.AluOpType
BF = ml_dtypes.bfloat16

B, T, D = 2, 2048, 1024
H, DH = 16, 64
R = B * T          # 4096 flattened rows
NCORE = 8
RC = R // NCORE    # 512 rows per core
EPS = 1e-5
D3 = 3 * D         # 3072
DF = 4 * D         # 4096
VA = H * (DH + 1)  # 1040: V augmented with a ones column per head
NKT = T // 128     # 16 key tiles
NDIAG = 4          # diagonal key tiles per core (512 q rows / 128)
NEG = -1.0e9


def _slot_order(qb128):
    """Key-tile processing order for a core whose queries start at 128*qb128.
    Slots 0..3 = the diagonal tiles; slots 4..15 = the rest ascending."""
    diag = list(range(qb128, qb128 + NDIAG))
    rest = [k for k in range(NKT) if k not in diag]
    return diag + rest


def _layer_norm_tiles(nc, sb, x_tile, out_dtype):
    """Plain LN (no scale/shift: gamma/beta are folded into the downstream
    weights host-side) over free axis of x_tile [128, D] fp32 -> [128, D]."""
    stats = sb.tile([128, 2, 6], F32, tag="ln_stats")
    xg = x_tile[:].rearrange("p (s d) -> p s d", s=2)
    for s in range(2):
        nc.vector.bn_stats(stats[:, s, :], xg[:, s, :])
    mv = sb.tile([128, 2], F32, tag="ln_mv")
    nc.vector.bn_aggr(mv[:], stats[:])
    rstd = sb.tile([128, 1], F32, tag="ln_rstd")
    eps_sb = sb.tile([128, 1], F32, tag="ln_eps")
    nc.vector.memset(eps_sb[:], EPS)
    nc.scalar.activation(rstd[:], mv[:, 1:2], AF.Sqrt, bias=eps_sb[:], scale=1.0)
    nc.vector.reciprocal(rstd[:], rstd[:])
    out = sb.tile([128, D], out_dtype, tag="ln_out")
    nc.vector.tensor_scalar(
        out[:], x_tile[:], mv[:, 0:1], rstd[:], ALU.subtract, ALU.mult,
    )
    return out


def _pe_transpose(nc, tps, dst_tiles, src_tile, t, ident):
    """Transpose src [128, D] bf16 into dst_tiles[j][:, t*128:(t+1)*128]
    via the PE transpose path + DVE copy out of PSUM."""
    for j in range(8):
        tp = tps.tile([128, 128], BF16, tag="tp")
        nc.tensor.transpose(tp[:], src_tile[:, j * 128:(j + 1) * 128], ident[:])
        if j % 2 == 0:
            nc.vector.tensor_copy(dst_tiles[j][:, t * 128:(t + 1) * 128], tp[:])
        else:
            # scalar engine is idle during the LN phases; share the copy-out
            nc.scalar.copy(dst_tiles[j][:, t * 128:(t + 1) * 128], tp[:])


def _build_a():
    nc = bacc.Bacc("TRN2", target_bir_lowering=False, debug=False, num_devices=NCORE)
    x = nc.dram_tensor("x", [RC, D], F32, kind="ExternalInput")
    wattn = nc.dram_tensor("wattn", [D, D3], BF16, kind="ExternalInput")
    battn = nc.dram_tensor("battn", [D3], F32, kind="ExternalInput")
    qkvT = nc.dram_tensor("qkvT", [D3, RC], BF16, kind="ExternalOutput")

    with tile.TileContext(nc) as tc:
        with (
            tc.tile_pool(name="const", bufs=1) as const,
            tc.tile_pool(name="w", bufs=1) as wpool,
            tc.tile_pool(name="sb", bufs=2) as sb,
            tc.tile_pool(name="lt", bufs=1) as lt,
            tc.tile_pool(name="ps", bufs=4, space="PSUM") as ps,
            tc.tile_pool(name="tps", bufs=3, space="PSUM") as tps,
            tc.tile_pool(name="outp", bufs=3) as outp,
        ):
            # x first (critical path: LN1 -> transpose -> matmul); weights
            # staggered on sync/scalar rings behind it.
            x_sb = [sb.tile([128, D], F32, tag=f"x{t}", name=f"x{t}") for t in range(4)]
            for t in range(4):
                nc.gpsimd.dma_start(x_sb[t][:], x[t * 128:(t + 1) * 128, :])

            ident = const.tile([128, 128], BF16, tag="ident")
            masks.make_identity(nc, ident[:])
            battn_all = const.tile([128, D3 // 128], F32, tag="battn")
            nc.gpsimd.dma_start(battn_all[:], battn.ap().rearrange("(m p) -> p m", p=128))

            w_sb = [wpool.tile([128, D3], BF16, tag=f"w{k}", name=f"w{k}") for k in range(8)]
            for k in range(8):
                eng = nc.sync if k % 2 == 0 else nc.gpsimd
                eng.dma_start(w_sb[k][:], wattn[k * 128:(k + 1) * 128, :])

            ln1T = [lt.tile([128, RC], BF16, tag=f"ln1T{j}", name=f"ln1T{j}") for j in range(8)]
            for t in range(4):
                ln1n = _layer_norm_tiles(nc, sb, x_sb[t], BF16)
                _pe_transpose(nc, tps, ln1T, ln1n, t, ident)

            for m in range(D3 // 128):  # 24 output col-tiles
                psum = ps.tile([128, RC], F32, tag="mm")
                for k in range(8):
                    nc.tensor.matmul(
                        psum[:], w_sb[k][:, m * 128:(m + 1) * 128], ln1T[k][:],
                        start=(k == 0), stop=(k == 7),
                    )
                o_sb = outp.tile([128, RC], BF16, tag="o")
                nc.scalar.activation(
                    o_sb[:], psum[:], AF.Identity,
                    bias=battn_all[:, m:m + 1], scale=1.0,
                )
                nc.sync.dma_start(qkvT[m * 128:(m + 1) * 128, :], o_sb[:])

    nc.compile()
    return nc


def _build_b():
    nc = bacc.Bacc("TRN2", target_bir_lowering=False, debug=False, num_devices=NCORE)
    x = nc.dram_tensor("x", [RC, D], F32, kind="ExternalInput")
    qT = nc.dram_tensor("qT", [D, RC], BF16, kind="ExternalInput")
    kT = nc.dram_tensor("kT", [D, T], BF16, kind="ExternalInput")          # slot-permuted
    vaug = nc.dram_tensor("vaug", [T, VA], BF16, kind="ExternalInput")     # slot-permuted
    biast = nc.dram_tensor("biast", [128, NKT - NDIAG], F32, kind="ExternalInput")
    diagmask = nc.dram_tensor("diagmask", [NDIAG * 128, 2 * RC], BF16, kind="ExternalInput")
    wproj = nc.dram_tensor("wproj", [D, D], BF16, kind="ExternalInput")
    bproj = nc.dram_tensor("bproj", [D], F32, kind="ExternalInput")
    wfc = nc.dram_tensor("wfc", [D, DF], BF16, kind="ExternalInput")
    bfc = nc.dram_tensor("bfc", [DF], F32, kind="ExternalInput")
    wout = nc.dram_tensor("wout", [DF, D], BF16, kind="ExternalInput")
    bout = nc.dram_tensor("bout", [D], F32, kind="ExternalInput")
    out = nc.dram_tensor("out", [RC, D], F32, kind="ExternalOutput")

    with tile.TileContext(nc) as tc:
        with (
            tc.tile_pool(name="const", bufs=1) as const,
            tc.tile_pool(name="att", bufs=1) as attp,
            tc.tile_pool(name="sb", bufs=2) as sb,
        ):
            attT = [attp.tile([128, RC], BF16, tag=f"attT{i}", name=f"attT{i}") for i in range(8)]

            # ---------------- attention ----------------
            with (
                tc.tile_pool(name="kv", bufs=1) as kv,
                tc.tile_pool(name="exps", bufs=20) as exps,
                tc.tile_pool(name="aps", bufs=3, space="PSUM") as aps,
                tc.tile_pool(name="avps", bufs=2, space="PSUM") as avps,
                tc.tile_pool(name="asb", bufs=4) as asb,
            ):
                # critical-path DMAs first: qT/kT per head pair on sync,
                # vaug slots + tiny mask/bias tables early on scalar/gpsimd.
                qT_sb = [kv.tile([128, RC], BF16, tag=f"qT{i}", name=f"qTs{i}") for i in range(8)]
                kT_sb = [kv.tile([128, T], BF16, tag=f"kT{i}", name=f"kTs{i}") for i in range(8)]
                v_sb = [kv.tile([128, VA], BF16, tag=f"v{i}", name=f"vs{i}") for i in range(NKT)]
                dm_sb = [kv.tile([128, 2 * RC], BF16, tag=f"dm{i}", name=f"dms{i}")
                         for i in range(NDIAG)]
                biast_sb = kv.tile([128, NKT - NDIAG], F32, tag="biast")
                nc.gpsimd.dma_start(biast_sb[:], biast[:, :])
                for i in range(NDIAG):
                    nc.gpsimd.dma_start(dm_sb[i][:], diagmask[i * 128:(i + 1) * 128, :])
                for i in range(8):
                    nc.sync.dma_start(qT_sb[i][:], qT[i * 128:(i + 1) * 128, :])
                    nc.sync.dma_start(kT_sb[i][:], kT[i * 128:(i + 1) * 128, :])
                for i in range(NKT):
                    nc.gpsimd.dma_start(v_sb[i][:], vaug[i * 128:(i + 1) * 128, :])

                # weights / residual stream loads overlap the attention phase
                ident = const.tile([128, 128], BF16, tag="ident")
                masks.make_identity(nc, ident[:])
                bproj_bc = const.tile([128, D], F32, tag="bproj")
                nc.gpsimd.dma_start(bproj_bc[:], bproj.ap().partition_broadcast(128))
                bout_bc = const.tile([128, D], F32, tag="bout")
                nc.gpsimd.dma_start(bout_bc[:], bout.ap().partition_broadcast(128))
                bfc_all = const.tile([128, DF // 128], F32, tag="bfc")
                nc.gpsimd.dma_start(bfc_all[:], bfc.ap().rearrange("(m p) -> p m", p=128))

                for hp in range(H // 2):  # head pairs (2hp, 2hp+1)
                    e_tiles = []
                    for s in range(NKT):
                        s_ps = aps.tile([128, 2 * RC], F32, tag="s")
                        nc.tensor.matmul(
                            s_ps[:, 0:RC],
                            kT_sb[hp][0:64, s * 128:(s + 1) * 128],
                            qT_sb[hp][0:64, :],
                            start=True, stop=True, tile_position=(0, 0),
                        )
                        nc.tensor.matmul(
                            s_ps[:, RC:2 * RC],
                            kT_sb[hp][64:128, s * 128:(s + 1) * 128],
                            qT_sb[hp][64:128, :],
                            start=True, stop=True, tile_position=(64, 0),
                        )
                        e_sb = exps.tile([128, 2 * RC], BF16, tag="e")
                        if s < NDIAG:
                            # diagonal tile: plain exp then static triangular mask
                            nc.scalar.activation(e_sb[:], s_ps[:], AF.Exp,
                                                 bias=0.0, scale=0.125)
                            nc.vector.tensor_mul(e_sb[:], e_sb[:], dm_sb[s][:])
                        else:
                            # interior tile: bias is 0 (fully visible) or -1e9
                            # (fully masked -> exp gives exact 0), per-core data
                            nc.scalar.activation(e_sb[:], s_ps[:], AF.Exp,
                                                 bias=biast_sb[:, s - NDIAG:s - NDIAG + 1],
                                                 scale=0.125)
                        e_tiles.append(e_sb)
                    psums = asb.tile([1, 2 * RC], F32, tag="psums")
                    for half in range(2):
                        h = 2 * hp + half
                        hl = half * 64
                        av_ps = avps.tile([DH + 1, RC], F32, tag="av")
                        for s in range(NKT):
                            nc.tensor.matmul(
                                av_ps[:], v_sb[s][:, h * 65:(h + 1) * 65],
                                e_tiles[s][:, half * RC:(half + 1) * RC],
                                start=(s == 0), stop=(s == NKT - 1),
                            )
                        nc.vector.tensor_copy(
                            psums[0:1, half * RC:(half + 1) * RC],
                            av_ps[DH:DH + 1, :])
                        nc.vector.tensor_copy(attT[hp][hl:hl + 64, :], av_ps[0:DH, :])
                    # normalize this pair now; overlaps next pair's QK/exp
                    recips = asb.tile([1, 2 * RC], F32, tag="recips")
                    nc.vector.reciprocal_approx_fast(recips[:], psums[:])
                    r16 = asb.tile([1, 2 * RC], BF16, tag="r16")
                    with nc.allow_low_precision(reason="softmax recip in bf16"):
                        nc.vector.tensor_copy(r16[:], recips[:])
                    rb_sb = asb.tile([128, 2 * RC], BF16, tag="rb")
                    nc.gpsimd.partition_broadcast(rb_sb[:], r16[:])
                    nc.vector.tensor_mul(
                        attT[hp][0:64, :], attT[hp][0:64, :], rb_sb[0:64, 0:RC],
                    )
                    nc.vector.tensor_mul(
                        attT[hp][64:128, :], attT[hp][64:128, :],
                        rb_sb[64:128, RC:2 * RC],
                    )

            # ---------------- proj + residual + LN2 ----------------
            x2p_cm = tc.tile_pool(name="x2p", bufs=1)
            x2p = x2p_cm.__enter__()
            x_sb = [x2p.tile([128, D], F32, tag=f"x{t}", name=f"x{t}") for t in range(4)]
            for t in range(4):
                nc.gpsimd.dma_start(x_sb[t][:], x[t * 128:(t + 1) * 128, :])
                # fold the proj bias into the residual ahead of time (gpsimd,
                # off the critical chain): x_sb becomes x + bproj
                nc.gpsimd.tensor_add(x_sb[t][:], x_sb[t][:], bproj_bc[:])
            ln2T = [x2p.tile([128, RC], BF16, tag=f"ln2T{j}", name=f"ln2T{j}") for j in range(8)]
            x2_sb = [x2p.tile([128, D], F32, tag=f"x2{t}", name=f"x2{t}") for t in range(4)]
            with (
                tc.tile_pool(name="wpj", bufs=1) as wpj,
                tc.tile_pool(name="pps", bufs=3, space="PSUM") as pps,
                tc.tile_pool(name="tps", bufs=3, space="PSUM") as tps,
            ):
                wp_sb = [wpj.tile([128, D], BF16, tag=f"wp{i}", name=f"wp{i}") for i in range(8)]
                for i in range(8):
                    nc.sync.dma_start(wp_sb[i][:], wproj[i * 128:(i + 1) * 128, :])
                for qm in range(4):
                    for oc in range(2):
                        y_ps = pps.tile([128, 512], F32, tag="y")
                        for cc in range(8):
                            nc.tensor.matmul(
                                y_ps[:],
                                attT[cc][:, qm * 128:(qm + 1) * 128],
                                wp_sb[cc][:, oc * 512:(oc + 1) * 512],
                                start=(cc == 0), stop=(cc == 7),
                            )
                        sl = slice(oc * 512, (oc + 1) * 512)
                        nc.vector.tensor_add(x2_sb[qm][:, sl], y_ps[:], x_sb[qm][:, sl])
                    ln2n = _layer_norm_tiles(nc, sb, x2_sb[qm], BF16)
                    _pe_transpose(nc, tps, ln2T, ln2n, qm, ident)
                    # pre-bias the FFN residual in place (WAR on LN2 reads is
                    # tracked by the tile framework)
                    nc.gpsimd.tensor_add(x2_sb[qm][:], x2_sb[qm][:], bout_bc[:])

            # ---------------- FFN ----------------
            with tc.tile_pool(name="g", bufs=1) as gp:
                g_sb = [gp.tile([128, RC], BF16, tag=f"g{i}", name=f"g{i}") for i in range(32)]
                woA_cm = tc.tile_pool(name="woA", bufs=1)
                woA = woA_cm.__enter__()
                # first half of w_out preloads while wfc is still resident
                wo_sb = [woA.tile([128, D], BF16, tag=f"wo{i}", name=f"wos{i}")
                         for i in range(16)]
                for i in range(16):
                    eng = nc.sync if i % 2 == 0 else nc.gpsimd
                    eng.dma_start(wo_sb[i][:], wout[i * 128:(i + 1) * 128, :])
                with (
                    tc.tile_pool(name="wf", bufs=1) as wf,
                    tc.tile_pool(name="fps", bufs=4, space="PSUM") as fps,
                ):
                    wf_sb = [wf.tile([128, DF], BF16, tag=f"wf{i}", name=f"wfs{i}") for i in range(8)]
                    for i in range(8):
                        eng = nc.sync if i % 2 == 0 else nc.gpsimd
                        eng.dma_start(wf_sb[i][:], wfc[i * 128:(i + 1) * 128, :])
                    for hm in range(32):
                        h_ps = fps.tile([128, RC], F32, tag="h")
                        for k in range(8):
                            nc.tensor.matmul(
                                h_ps[:], wf_sb[k][:, hm * 128:(hm + 1) * 128],
                                ln2T[k][:], start=(k == 0), stop=(k == 7),
                            )
                        nc.scalar.activation(
                            g_sb[hm][:], h_ps[:], AF.Gelu,
                            bias=bfc_all[:, hm:hm + 1], scale=1.0,
                        )

                with (
                    tc.tile_pool(name="woB", bufs=1) as woB,
                    tc.tile_pool(name="ops", bufs=4, space="PSUM") as ops,
                    tc.tile_pool(name="osb", bufs=3) as osb,
                ):
                    wo_sb += [woB.tile([128, D], BF16, tag=f"wo{i}", name=f"wosb{i}")
                              for i in range(16, 32)]
                    for i in range(16, 32):
                        eng = nc.sync if i % 2 == 0 else nc.gpsimd
                        eng.dma_start(wo_sb[i][:], wout[i * 128:(i + 1) * 128, :])
                    for qm in range(4):
                        o_tile = osb.tile([128, D], F32, tag="o")
                        o_ps = [ops.tile([128, 512], F32, tag="ops", name=f"ops{qm}_{i}")
                                for i in range(2)]
                        for hh in range(32):
                            for oc in range(2):
                                nc.tensor.matmul(
                                    o_ps[oc][:],
                                    g_sb[hh][:, qm * 128:(qm + 1) * 128],
                                    wo_sb[hh][:, oc * 512:(oc + 1) * 512],
                                    start=(hh == 0), stop=(hh == 31),
                                )
                        for oc in range(2):
                            sl = slice(oc * 512, (oc + 1) * 512)
                            nc.vector.tensor_add(o_tile[:, sl], o_ps[oc][:], x2_sb[qm][:, sl])
                        nc.sync.dma_start(out[qm * 128:(qm + 1) * 128, :], o_tile[:])
                woA_cm.__exit__(None, None, None)
            x2p_cm.__exit__(None, None, None)

    nc.compile()
    return nc


_CACHE = {}


def _get(name, builder):
    if name not in _CACHE:
        _CACHE[name] = builder()
    return _CACHE[name]


def _prep_a_inputs(inputs, xf):
    # fold ln1 gamma/beta into the qkv projection: ln1(x)@W + b =
    # xn@(diag(g)W) + (beta@W + b)
    w = np.asarray(inputs["w_attn"], np.float32)
    g = np.asarray(inputs["ln1_w"], np.float32)
    beta = np.asarray(inputs["ln1_b"], np.float32)
    wattn_bf = (g[:, None] * w).astype(BF)
    battn = np.asarray(inputs["b_attn"], np.float32) + beta @ w
    return [
        dict(x=xf[c * RC:(c + 1) * RC], wattn=wattn_bf, battn=battn)
        for c in range(NCORE)
    ]


def _diagmask():
    """Static triangular masks for slots 0..3: keep where q >= p + 128*s.
    [4*128, 2*RC] bf16, the [128, RC] pattern duplicated for both packed heads."""
    p = np.arange(128)
    q = np.arange(RC)
    dm = np.empty((NDIAG * 128, 2 * RC), dtype=BF)
    for s in range(NDIAG):
        m = (q[None, :] >= (p[:, None] + 128 * s)).astype(BF)
        dm[s * 128:(s + 1) * 128, 0:RC] = m
        dm[s * 128:(s + 1) * 128, RC:2 * RC] = m
    return dm


def _prep_b_inputs(inputs, xf, qkvT):
    """Host reassembly: full K^T/V per batch, per-core slot permutation."""
    kT_b = [np.concatenate([qkvT[4 * b + r][D:2 * D] for r in range(4)], axis=1)
            for b in range(B)]                            # [1024, 2048] bf16
    vT_b = [np.concatenate([qkvT[4 * b + r][2 * D:3 * D] for r in range(4)], axis=1)
            for b in range(B)]
    vaug_b = []
    for b in range(B):
        vn = np.ascontiguousarray(vT_b[b].T)              # [2048, 1024]
        va = np.empty((T, H, DH + 1), dtype=BF)
        va[:, :, :DH] = vn.reshape(T, H, DH)
        va[:, :, DH] = np.ones((), dtype=BF)
        vaug_b.append(va.reshape(T, VA))

    dm = _diagmask()
    # fold ln2 gamma/beta into the fc projection (same identity as ln1)
    wf = np.asarray(inputs["w_fc"], np.float32)
    g2 = np.asarray(inputs["ln2_w"], np.float32)
    beta2 = np.asarray(inputs["ln2_b"], np.float32)
    wfc_bf = (g2[:, None] * wf).astype(BF)
    bfc = np.asarray(inputs["b_fc"], np.float32) + beta2 @ wf
    wproj_bf = np.asarray(inputs["w_proj"], np.float32).astype(BF)
    bproj = np.asarray(inputs["b_proj"], np.float32)
    wout_bf = np.asarray(inputs["w_out"], np.float32).astype(BF)
    bout = np.asarray(inputs["b_out"], np.float32)
    in_maps = []
    for c in range(NCORE):
        b = c // 4
        qb128 = (c % 4) * 4                               # first diagonal key tile
        order = _slot_order(qb128)
        # permute key tiles into slot order (columns of kT, rows of vaug)
        kTp = np.concatenate(
            [kT_b[b][:, k * 128:(k + 1) * 128] for k in order], axis=1)
        vap = np.concatenate(
            [vaug_b[b][k * 128:(k + 1) * 128] for k in order], axis=0)
        # exp bias for interior slots: 0 if fully visible, -1e9 if fully masked
        bias = np.zeros((128, NKT - NDIAG), np.float32)
        for j, k in enumerate(order[NDIAG:]):
            if k > qb128:                                 # above the diagonal band
                bias[:, j] = NEG
        in_maps.append(dict(
            x=xf[c * RC:(c + 1) * RC],
            qT=np.ascontiguousarray(qkvT[c][0:D]),
            kT=np.ascontiguousarray(kTp),
            vaug=np.ascontiguousarray(vap),
            biast=bias,
            diagmask=dm,
            wproj=wproj_bf, bproj=bproj, wfc=wfc_bf, bfc=bfc,
            wout=wout_bf, bout=bout,
        ))
    return in_maps


def kernel(x, ln1_w, ln1_b, ln2_w, ln2_b, w_attn, b_attn, w_proj, b_proj,
           w_fc, b_fc, w_out, b_out):
    inputs = dict(x=x, ln1_w=ln1_w, ln1_b=ln1_b, ln2_w=ln2_w, ln2_b=ln2_b,
                  w_attn=w_attn, b_attn=b_attn, w_proj=w_proj, b_proj=b_proj,
                  w_fc=w_fc, b_fc=b_fc, w_out=w_out, b_out=b_out)
    xf = np.ascontiguousarray(np.asarray(x, np.float32).reshape(R, D))
    cores = list(range(NCORE))

    nc_a = _get("a", _build_a)
    res_a = run_bass_kernel_spmd(nc_a, _prep_a_inputs(inputs, xf), cores).results
    qkvT = [np.asarray(res_a[c]["qkvT"]) for c in cores]  # [3072, 512] bf16

    nc_b = _get("b", _build_b)
    in_maps_b = _prep_b_inputs(inputs, xf, qkvT)
    res_b = run_bass_kernel_spmd(nc_b, in_maps_b, cores).results
    out = np.concatenate([np.asarray(res_b[c]["out"], np.float32) for c in cores], axis=0)
    return out.reshape(B, T, D)


# revision 36
# speedup vs baseline: 1.0924x; 1.0631x over previous
"""Trainium2 Bass kernel for a GPT-style transformer block.

Shapes (hardcoded): x [2, 2048, 1024], n_head=16, causal attention + GELU MLP.
Strategy: row-sharding (4096 rows -> 512 rows/core on 8 cores).
  Launch A: per-core LN1 + qkv projection for own rows -> qkvT [3072, 512] bf16.
  Host:     reassemble full K^T / V per batch; per-core SLOT PERMUTATION of the
            16 key tiles so the 4 diagonal tiles sit at fixed slots 0-3 (their
            triangular masks are then core-independent constants), remaining
            slots carry a per-core {0,-1e9} exp bias that zeroes fully-masked
            tiles for free inside the activation.
  Launch B: per-core attention over own 512 query rows, proj, residual, LN2,
            FFN, residual -> out rows [512, 1024].
All matmuls bf16 with fp32 PSUM accumulation; residual stream / LN / softmax
sums fp32. Attention computes scores for head pairs row-packed on the PE
(K=64 halves at tile_position (0,0)/(64,0)) into one [128, 1024] PSUM span so
exp runs on big tiles (scalar ACTIVATE cost is (N+352)/1.2ns, N = free size).
"""

import sys

if "/opt/trn_rl_repo" not in sys.path:
    sys.path.insert(0, "/opt/trn_rl_repo")

import numpy as np
import ml_dtypes

import concourse.bacc as bacc
import concourse.tile as tile
from concourse import masks
from concourse import mybir
from concourse.bass_utils import run_bass_kernel_spmd

F32 = mybir.dt.float32
BF16 = mybir.dt.bfloat16
AF = mybir.ActivationFunctionType
ALU = mybir.AluOpType
BF = ml_dtypes.bfloat16

B, T, D = 2, 2048, 1024
H, DH = 16, 64
R = B * T          # 4096 flattened rows
NCORE = 8
RC = R // NCORE    # 512 rows per core
EPS = 1e-5
D3 = 3 * D         # 3072
DF = 4 * D         # 4096
VA = H * (DH + 1)  # 1040: V augmented with a ones column per head
NKT = T // 128     # 16 key tiles
NDIAG = 4          # diagonal key tiles per core (512 q rows / 128)
NEG = -1.0e9


def _slot_order(qb128):
    """Key-tile processing order for a core whose queries start at 128*qb128.
    Slots 0..3 = the diagonal tiles; slots 4..15 = the rest ascending."""
    diag = list(range(qb128, qb128 + NDIAG))
    rest = [k for k in range(NKT) if k not in diag]
    return diag + rest


def _layer_norm_tiles(nc, sb, x_tile, out_dtype):
    """Plain LN (no scale/shift: gamma/beta are folded into the downstream
    weights host-side) over free axis of x_tile [128, D] fp32 -> [128, D]."""
    stats = sb.tile([128, 2, 6], F32, tag="ln_stats")
    xg = x_tile[:].rearrange("p (s d) -> p s d", s=2)
    for s in range(2):
        nc.vector.bn_stats(stats[:, s, :], xg[:, s, :])
    mv = sb.tile([128, 2], F32, tag="ln_mv")
    nc.vector.bn_aggr(mv[:], stats[:])
    rstd = sb.tile([128, 1], F32, tag="ln_rstd")
    eps_sb = sb.tile([128, 1], F32, tag="ln_eps")
    nc.vector.memset(eps_sb[:], EPS)
    nc.scalar.activation(rstd[:], mv[:, 1:2], AF.Sqrt, bias=eps_sb[:], scale=1.0)
    nc.vector.reciprocal(rstd[:], rstd[:])
    out = sb.tile([128, D], out_dtype, tag="ln_out")
    nc.vector.tensor_scalar(
        out[:], x_tile[:], mv[:, 0:1], rstd[:], ALU.subtract, ALU.mult,
    )
    return out


def _pe_transpose(nc, tps, dst_tiles, src_tile, t, ident):
    """Transpose src [128, D] bf16 into dst_tiles[j][:, t*128:(t+1)*128]
    via the PE transpose path + DVE copy out of PSUM."""
    for j in range(8):
        tp = tps.tile([128, 128], BF16, tag="tp")
        nc.tensor.transpose(tp[:], src_tile[:, j * 128:(j + 1) * 128], ident[:])
        if j % 2 == 0:
            nc.vector.tensor_copy(dst_tiles[j][:, t * 128:(t + 1) * 128], tp[:])
        else:
            # scalar engine is idle during the LN phases; share the copy-out
            nc.scalar.copy(dst_tiles[j][:, t * 128:(t + 1) * 128], tp[:])


def _build_a():
    nc = bacc.Bacc("TRN2", target_bir_lowering=False, debug=False, num_devices=NCORE)
    x = nc.dram_tensor("x", [RC, D], F32, kind="ExternalInput")
    wattn = nc.dram_tensor("wattn", [D, D3], BF16, kind="ExternalInput")
    battn = nc.dram_tensor("battn", [D3], F32, kind="ExternalInput")
    qkvT = nc.dram_tensor("qkvT", [D3, RC], BF16, kind="ExternalOutput")

    with tile.TileContext(nc) as tc:
        with (
            tc.tile_pool(name="const", bufs=1) as const,
            tc.tile_pool(name="w", bufs=1) as wpool,
            tc.tile_pool(name="sb", bufs=2) as sb,
            tc.tile_pool(name="lt", bufs=1) as lt,
            tc.tile_pool(name="ps", bufs=4, space="PSUM") as ps,
            tc.tile_pool(name="tps", bufs=3, space="PSUM") as tps,
            tc.tile_pool(name="outp", bufs=3) as outp,
        ):
            # x first (critical path: LN1 -> transpose -> matmul); weights
            # staggered on sync/scalar rings behind it.
            x_sb = [sb.tile([128, D], F32, tag=f"x{t}", name=f"x{t}") for t in range(4)]
            for t in range(4):
                nc.gpsimd.dma_start(x_sb[t][:], x[t * 128:(t + 1) * 128, :])

            ident = const.tile([128, 128], BF16, tag="ident")
            masks.make_identity(nc, ident[:])
            battn_all = const.tile([128, D3 // 128], F32, tag="battn")
            nc.gpsimd.dma_start(battn_all[:], battn.ap().rearrange("(m p) -> p m", p=128))

            w_sb = [wpool.tile([128, D3], BF16, tag=f"w{k}", name=f"w{k}") for k in range(8)]
            for k in range(8):
                eng = nc.sync if k % 2 == 0 else nc.gpsimd
                eng.dma_start(w_sb[k][:], wattn[k * 128:(k + 1) * 128, :])

            ln1T = [lt.tile([128, RC], BF16, tag=f"ln1T{j}", name=f"ln1T{j}") for j in range(8)]
            for t in range(4):
                ln1n = _layer_norm_tiles(nc, sb, x_sb[t], BF16)
                _pe_transpose(nc, tps, ln1T, ln1n, t, ident)

            for m in range(D3 // 128):  # 24 output col-tiles
                psum = ps.tile([128, RC], F32, tag="mm")
                for k in range(8):
                    nc.tensor.matmul(
                        psum[:], w_sb[k][:, m * 128:(m + 1) * 128], ln1T[k][:],
                        start=(k == 0), stop=(k == 7),
                    )
                o_sb = outp.tile([128, RC], BF16, tag="o")
                nc.scalar.activation(
                    o_sb[:], psum[:], AF.Identity,
                    bias=battn_all[:, m:m + 1], scale=1.0,
                )
                nc.sync.dma_start(qkvT[m * 128:(m + 1) * 128, :], o_sb[:])

    nc.compile()
    return nc


def _build_b():
    nc = bacc.Bacc("TRN2", target_bir_lowering=False, debug=False, num_devices=NCORE)
    x = nc.dram_tensor("x", [RC, D], F32, kind="ExternalInput")
    qT = nc.dram_tensor("qT", [D, RC], BF16, kind="ExternalInput")
    kT = nc.dram_tensor("kT", [D, T], BF16, kind="ExternalInput")          # slot-permuted
    vaug = nc.dram_tensor("vaug", [T, VA], BF16, kind="ExternalInput")     # slot-permuted
    biast = nc.dram_tensor("biast", [128, NKT - NDIAG], F32, kind="ExternalInput")
    diagmask = nc.dram_tensor("diagmask", [NDIAG * 128, 2 * RC], BF16, kind="ExternalInput")
    wproj = nc.dram_tensor("wproj", [D, D], BF16, kind="ExternalInput")
    bproj = nc.dram_tensor("bproj", [D], F32, kind="ExternalInput")
    wfc = nc.dram_tensor("wfc", [D, DF], BF16, kind="ExternalInput")
    bfc = nc.dram_tensor("bfc", [DF], F32, kind="ExternalInput")
    wout = nc.dram_tensor("wout", [DF, D], BF16, kind="ExternalInput")
    bout = nc.dram_tensor("bout", [D], F32, kind="ExternalInput")
    out = nc.dram_tensor("out", [RC, D], F32, kind="ExternalOutput")

    with tile.TileContext(nc) as tc:
        with (
            tc.tile_pool(name="const", bufs=1) as const,
            tc.tile_pool(name="att", bufs=1) as attp,
            tc.tile_pool(name="sb", bufs=2) as sb,
        ):
            attT = [attp.tile([128, RC], BF16, tag=f"attT{i}", name=f"attT{i}") for i in range(8)]

            # ---------------- attention ----------------
            with (
                tc.tile_pool(name="kv", bufs=1) as kv,
                tc.tile_pool(name="exps", bufs=20) as exps,
                tc.tile_pool(name="aps", bufs=3, space="PSUM") as aps,
                tc.tile_pool(name="avps", bufs=2, space="PSUM") as avps,
                tc.tile_pool(name="asb", bufs=4) as asb,
            ):
                # critical-path DMAs first: qT/kT per head pair on sync,
                # vaug slots + tiny mask/bias tables early on scalar/gpsimd.
                qT_sb = [kv.tile([128, RC], BF16, tag=f"qT{i}", name=f"qTs{i}") for i in range(8)]
                kT_sb = [kv.tile([128, T], BF16, tag=f"kT{i}", name=f"kTs{i}") for i in range(8)]
                v_sb = [kv.tile([128, VA], BF16, tag=f"v{i}", name=f"vs{i}") for i in range(NKT)]
                dm_sb = [kv.tile([128, 2 * RC], BF16, tag=f"dm{i}", name=f"dms{i}")
                         for i in range(NDIAG)]
                biast_sb = kv.tile([128, NKT - NDIAG], F32, tag="biast")
                nc.gpsimd.dma_start(biast_sb[:], biast[:, :])
                for i in range(NDIAG):
                    nc.gpsimd.dma_start(dm_sb[i][:], diagmask[i * 128:(i + 1) * 128, :])
                for i in range(8):
                    nc.sync.dma_start(qT_sb[i][:], qT[i * 128:(i + 1) * 128, :])
                    nc.sync.dma_start(kT_sb[i][:], kT[i * 128:(i + 1) * 128, :])
                for i in range(NKT):
                    nc.gpsimd.dma_start(v_sb[i][:], vaug[i * 128:(i + 1) * 128, :])

                # weights / residual stream loads overlap the attention phase
                ident = const.tile([128, 128], BF16, tag="ident")
                masks.make_identity(nc, ident[:])
                bproj_bc = const.tile([128, D], F32, tag="bproj")
                nc.gpsimd.dma_start(bproj_bc[:], bproj.ap().partition_broadcast(128))
                bout_bc = const.tile([128, D], F32, tag="bout")
                nc.gpsimd.dma_start(bout_bc[:], bout.ap().partition_broadcast(128))
                bfc_all = const.tile([128, DF // 128], F32, tag="bfc")
                nc.gpsimd.dma_start(bfc_all[:], bfc.ap().rearrange("(m p) -> p m", p=128))

                def emit_scores(hp):
                    e_tiles = []
                    for s in range(NKT):
                        s_ps = aps.tile([128, 2 * RC], F32, tag="s", name=f"s{hp}_{s}")
                        nc.tensor.matmul(
                            s_ps[:, 0:RC],
                            kT_sb[hp][0:64, s * 128:(s + 1) * 128],
                            qT_sb[hp][0:64, :],
                            start=True, stop=True, tile_position=(0, 0),
                        )
                        nc.tensor.matmul(
                            s_ps[:, RC:2 * RC],
                            kT_sb[hp][64:128, s * 128:(s + 1) * 128],
                            qT_sb[hp][64:128, :],
                            start=True, stop=True, tile_position=(64, 0),
                        )
                        e_sb = exps.tile([128, 2 * RC], BF16, tag="e", name=f"e{hp}_{s}")
                        if s < NDIAG:
                            # diagonal tile: plain exp then static triangular mask
                            nc.scalar.activation(e_sb[:], s_ps[:], AF.Exp,
                                                 bias=0.0, scale=0.125)
                            nc.vector.tensor_mul(e_sb[:], e_sb[:], dm_sb[s][:])
                        else:
                            # interior tile: bias is 0 (fully visible) or -1e9
                            # (fully masked -> exp gives exact 0), per-core data
                            nc.scalar.activation(e_sb[:], s_ps[:], AF.Exp,
                                                 bias=biast_sb[:, s - NDIAG:s - NDIAG + 1],
                                                 scale=0.125)
                        e_tiles.append(e_sb)
                    return e_tiles

                def emit_av(hp, e_tiles):
                    psums = asb.tile([1, 2 * RC], F32, tag="psums", name=f"ps{hp}")
                    for half in range(2):
                        h = 2 * hp + half
                        hl = half * 64
                        av_ps = avps.tile([DH + 1, RC], F32, tag="av", name=f"av{hp}_{half}")
                        for s in range(NKT):
                            nc.tensor.matmul(
                                av_ps[:], v_sb[s][:, h * 65:(h + 1) * 65],
                                e_tiles[s][:, half * RC:(half + 1) * RC],
                                start=(s == 0), stop=(s == NKT - 1),
                            )
                        nc.vector.tensor_copy(
                            psums[0:1, half * RC:(half + 1) * RC],
                            av_ps[DH:DH + 1, :])
                        nc.vector.tensor_copy(attT[hp][hl:hl + 64, :], av_ps[0:DH, :])
                    recips = asb.tile([1, 2 * RC], F32, tag="recips", name=f"rc{hp}")
                    nc.vector.reciprocal_approx_fast(recips[:], psums[:])
                    r16 = asb.tile([1, 2 * RC], BF16, tag="r16", name=f"r16_{hp}")
                    with nc.allow_low_precision(reason="softmax recip in bf16"):
                        nc.vector.tensor_copy(r16[:], recips[:])
                    rb_sb = asb.tile([128, 2 * RC], BF16, tag="rb", name=f"rb{hp}")
                    nc.gpsimd.partition_broadcast(rb_sb[:], r16[:])
                    nc.vector.tensor_mul(
                        attT[hp][0:64, :], attT[hp][0:64, :], rb_sb[0:64, 0:RC],
                    )
                    nc.vector.tensor_mul(
                        attT[hp][64:128, :], attT[hp][64:128, :],
                        rb_sb[64:128, RC:2 * RC],
                    )

                #Software-pipelined: each pair's AV is emitted after the NEXT
                # pair's scores so the PE never blocks the scalar exp stream
                # at a pair boundary.
                prev = None
                for hp in range(H // 2):  # head pairs (2hp, 2hp+1)
                    e_tiles = emit_scores(hp)
                    if prev is not None:
                        emit_av(prev[0], prev[1])
                    prev = (hp, e_tiles)
                emit_av(prev[0], prev[1])

            # ---------------- proj + residual + LN2 ----------------
            x2p_cm = tc.tile_pool(name="x2p", bufs=1)
            x2p = x2p_cm.__enter__()
            x_sb = [x2p.tile([128, D], F32, tag=f"x{t}", name=f"x{t}") for t in range(4)]
            for t in range(4):
                nc.gpsimd.dma_start(x_sb[t][:], x[t * 128:(t + 1) * 128, :])
                # fold the proj bias into the residual ahead of time (gpsimd,
                # off the critical chain): x_sb becomes x + bproj
                nc.gpsimd.tensor_add(x_sb[t][:], x_sb[t][:], bproj_bc[:])
            ln2T = [x2p.tile([128, RC], BF16, tag=f"ln2T{j}", name=f"ln2T{j}") for j in range(8)]
            x2_sb = [x2p.tile([128, D], F32, tag=f"x2{t}", name=f"x2{t}") for t in range(4)]
            with (
                tc.tile_pool(name="wpj", bufs=1) as wpj,
                tc.tile_pool(name="pps", bufs=3, space="PSUM") as pps,
                tc.tile_pool(name="tps", bufs=3, space="PSUM") as tps,
            ):
                wp_sb = [wpj.tile([128, D], BF16, tag=f"wp{i}", name=f"wp{i}") for i in range(8)]
                for i in range(8):
                    nc.sync.dma_start(wp_sb[i][:], wproj[i * 128:(i + 1) * 128, :])
                for qm in range(4):
                    for oc in range(2):
                        y_ps = pps.tile([128, 512], F32, tag="y")
                        for cc in range(8):
                            nc.tensor.matmul(
                                y_ps[:],
                                attT[cc][:, qm * 128:(qm + 1) * 128],
                                wp_sb[cc][:, oc * 512:(oc + 1) * 512],
                                start=(cc == 0), stop=(cc == 7),
                            )
                        sl = slice(oc * 512, (oc + 1) * 512)
                        nc.vector.tensor_add(x2_sb[qm][:, sl], y_ps[:], x_sb[qm][:, sl])
                    ln2n = _layer_norm_tiles(nc, sb, x2_sb[qm], BF16)
                    _pe_transpose(nc, tps, ln2T, ln2n, qm, ident)
                    # pre-bias the FFN residual in place (WAR on LN2 reads is
                    # tracked by the tile framework)
                    nc.gpsimd.tensor_add(x2_sb[qm][:], x2_sb[qm][:], bout_bc[:])

            # ---------------- FFN ----------------
            with tc.tile_pool(name="g", bufs=1) as gp:
                g_sb = [gp.tile([128, RC], BF16, tag=f"g{i}", name=f"g{i}") for i in range(32)]
                woA_cm = tc.tile_pool(name="woA", bufs=1)
                woA = woA_cm.__enter__()
                # first half of w_out preloads while wfc is still resident
                wo_sb = [woA.tile([128, D], BF16, tag=f"wo{i}", name=f"wos{i}")
                         for i in range(16)]
                for i in range(16):
                    eng = nc.sync if i % 2 == 0 else nc.gpsimd
                    eng.dma_start(wo_sb[i][:], wout[i * 128:(i + 1) * 128, :])
                with (
                    tc.tile_pool(name="wf", bufs=1) as wf,
                    tc.tile_pool(name="fps", bufs=4, space="PSUM") as fps,
                ):
                    wf_sb = [wf.tile([128, DF], BF16, tag=f"wf{i}", name=f"wfs{i}") for i in range(8)]
                    for i in range(8):
                        eng = nc.sync if i % 2 == 0 else nc.gpsimd
                        eng.dma_start(wf_sb[i][:], wfc[i * 128:(i + 1) * 128, :])
                    for hm in range(32):
                        h_ps = fps.tile([128, RC], F32, tag="h")
                        for k in range(8):
                            nc.tensor.matmul(
                                h_ps[:], wf_sb[k][:, hm * 128:(hm + 1) * 128],
                                ln2T[k][:], start=(k == 0), stop=(k == 7),
                            )
                        nc.scalar.activation(
                            g_sb[hm][:], h_ps[:], AF.Gelu,
                            bias=bfc_all[:, hm:hm + 1], scale=1.0,
                        )

                with (
                    tc.tile_pool(name="woB", bufs=1) as woB,
                    tc.tile_pool(name="ops", bufs=4, space="PSUM") as ops,
                    tc.tile_pool(name="osb", bufs=3) as osb,
                ):
                    wo_sb += [woB.tile([128, D], BF16, tag=f"wo{i}", name=f"wosb{i}")
                              for i in range(16, 32)]
                    for i in range(16, 32):
                        eng = nc.sync if i % 2 == 0 else nc.gpsimd
                        eng.dma_start(wo_sb[i][:], wout[i * 128:(i + 1) * 128, :])
                    for qm in range(4):
                        o_tile = osb.tile([128, D], F32, tag="o")
                        o_ps = [ops.tile([128, 512], F32, tag="ops", name=f"ops{qm}_{i}")
                                for i in range(2)]
                        for hh in range(32):
                            for oc in range(2):
                                nc.tensor.matmul(
                                    o_ps[oc][:],
                                    g_sb[hh][:, qm * 128:(qm + 1) * 128],
                                    wo_sb[hh][:, oc * 512:(oc + 1) * 512],
                                    start=(hh == 0), stop=(hh == 31),
                                )
                        for oc in range(2):
                            sl = slice(oc * 512, (oc + 1) * 512)
                            nc.vector.tensor_add(o_tile[:, sl], o_ps[oc][:], x2_sb[qm][:, sl])
                        nc.sync.dma_start(out[qm * 128:(qm + 1) * 128, :], o_tile[:])
                woA_cm.__exit__(None, None, None)
            x2p_cm.__exit__(None, None, None)

    nc.compile()
    return nc


_CACHE = {}


def _get(name, builder):
    if name not in _CACHE:
        _CACHE[name] = builder()
    return _CACHE[name]


def _prep_a_inputs(inputs, xf):
    # fold ln1 gamma/beta into the qkv projection: ln1(x)@W + b =
    # xn@(diag(g)W) + (beta@W + b)
    w = np.asarray(inputs["w_attn"], np.float32)
    g = np.asarray(inputs["ln1_w"], np.float32)
    beta = np.asarray(inputs["ln1_b"], np.float32)
    wattn_bf = (g[:, None] * w).astype(BF)
    battn = np.asarray(inputs["b_attn"], np.float32) + beta @ w
    return [
        dict(x=xf[c * RC:(c + 1) * RC], wattn=wattn_bf, battn=battn)
        for c in range(NCORE)
    ]


def _diagmask():
    """Static triangular masks for slots 0..3: keep where q >= p + 128*s.
    [4*128, 2*RC] bf16, the [128, RC] pattern duplicated for both packed heads."""
    p = np.arange(128)
    q = np.arange(RC)
    dm = np.empty((NDIAG * 128, 2 * RC), dtype=BF)
    for s in range(NDIAG):
        m = (q[None, :] >= (p[:, None] + 128 * s)).astype(BF)
        dm[s * 128:(s + 1) * 128, 0:RC] = m
        dm[s * 128:(s + 1) * 128, RC:2 * RC] = m
    return dm


def _prep_b_inputs(inputs, xf, qkvT):
    """Host reassembly: full K^T/V per batch, per-core slot permutation."""
    kT_b = [np.concatenate([qkvT[4 * b + r][D:2 * D] for r in range(4)], axis=1)
            for b in range(B)]                            # [1024, 2048] bf16
    vT_b = [np.concatenate([qkvT[4 * b + r][2 * D:3 * D] for r in range(4)], axis=1)
            for b in range(B)]
    vaug_b = []
    for b in range(B):
        vn = np.ascontiguousarray(vT_b[b].T)              # [2048, 1024]
        va = np.empty((T, H, DH + 1), dtype=BF)
        va[:, :, :DH] = vn.reshape(T, H, DH)
        va[:, :, DH] = np.ones((), dtype=BF)
        vaug_b.append(va.reshape(T, VA))

    dm = _diagmask()
    # fold ln2 gamma/beta into the fc projection (same identity as ln1)
    wf = np.asarray(inputs["w_fc"], np.float32)
    g2 = np.asarray(inputs["ln2_w"], np.float32)
    beta2 = np.asarray(inputs["ln2_b"], np.float32)
    wfc_bf = (g2[:, None] * wf).astype(BF)
    bfc = np.asarray(inputs["b_fc"], np.float32) + beta2 @ wf
    wproj_bf = np.asarray(inputs["w_proj"], np.float32).astype(BF)
    bproj = np.asarray(inputs["b_proj"], np.float32)
    wout_bf = np.asarray(inputs["w_out"], np.float32).astype(BF)
    bout = np.asarray(inputs["b_out"], np.float32)
    in_maps = []
    for c in range(NCORE):
        b = c // 4
        qb128 = (c % 4) * 4                               # first diagonal key tile
        order = _slot_order(qb128)
        # permute key tiles into slot order (columns of kT, rows of vaug)
        kTp = np.concatenate(
            [kT_b[b][:, k * 128:(k + 1) * 128] for k in order], axis=1)
        vap = np.concatenate(
            [vaug_b[b][k * 128:(k + 1) * 128] for k in order], axis=0)
        # exp bias for interior slots: 0 if fully visible, -1e9 if fully masked
        bias = np.zeros((128, NKT - NDIAG), np.float32)
        for j, k in enumerate(order[NDIAG:]):
            if k > qb128:                                 # above the diagonal band
                bias[:, j] = NEG
        in_maps.append(dict(
            x=xf[c * RC:(c + 1) * RC],
            qT=np.ascontiguousarray(qkvT[c][0:D]),
            kT=np.ascontiguousarray(kTp),
            vaug=np.ascontiguousarray(vap),
            biast=bias,
            diagmask=dm,
            wproj=wproj_bf, bproj=bproj, wfc=wfc_bf, bfc=bfc,
            wout=wout_bf, bout=bout,
        ))
    return in_maps


def kernel(x, ln1_w, ln1_b, ln2_w, ln2_b, w_attn, b_attn, w_proj, b_proj,
           w_fc, b_fc, w_out, b_out):
    inputs = dict(x=x, ln1_w=ln1_w, ln1_b=ln1_b, ln2_w=ln2_w, ln2_b=ln2_b,
                  w_attn=w_attn, b_attn=b_attn, w_proj=w_proj, b_proj=b_proj,
                  w_fc=w_fc, b_fc=b_fc, w_out=w_out, b_out=b_out)
    xf = np.ascontiguousarray(np.asarray(x, np.float32).reshape(R, D))
    cores = list(range(NCORE))

    nc_a = _get("a", _build_a)
    res_a = run_bass_kernel_spmd(nc_a, _prep_a_inputs(inputs, xf), cores).results
    qkvT = [np.asarray(res_a[c]["qkvT"]) for c in cores]  # [3072, 512] bf16

    nc_b = _get("b", _build_b)
    in_maps_b = _prep_b_inputs(inputs, xf, qkvT)
    res_b = run_bass_kernel_spmd(nc_b, in_maps_b, cores).results
    out = np.concatenate([np.asarray(res_b[c]["out"], np.float32) for c in cores], axis=0)
    return out.reshape(B, T, D)
